# revision 1
# baseline (speedup 1.0000x reference)
"""Trainium2 Bass kernel for BlockedMLP:
    h1 = relu(x @ w1.T + b1)            # dense fc1
    h2 = relu(bsr_linear(h1, W2_bsr))   # 64x64-blocked sparse fc2
    y  = h2 @ w3.T + b3                 # dense fc3

Strategy: data-parallel over the batch dim across 8 NeuronCores
(weights replicated, no collectives). Everything is computed in a
feature-major ("transposed") layout so matmuls contract over the
partition dim with N = Bsh = 512 batch columns:

    hT  [H, Bsh]   = W1 @ xT      (Bsh = 4096/8 = 512 batch rows/core)
    h2T [H, Bsh]   = W2 @ hT      (BSR: compile-time-known sparsity)
    yT  [Dout,Bsh] = W3 @ h2T

fc1/fc3 are dense 128x128xK matmul chains at full PE rate. fc2 uses the
"ladder" scheme: the PE array is split into 4 concurrent 64x64 tiles
via tile_position. Quadrant (kg, mg) = (rhs partition half, psum/out
partition half). Each of the 64 BSR block rows is assigned an output
half mg and accumulates ALL its blocks into one half PSUM bank via two
sequential chains (cols with parity kg=0, then kg=1 — order can differ
per row), so no cross-bank merge is needed. A host-side balance anneal
picks the col parity classes + row mg split so all 4 quadrant lanes
carry exactly nnz/4 blocks, and a bounded-open-rows greedy open-shop
schedule achieves the optimal makespan (528 slots for the canonical
mask) with at most 3 rows' PSUM banks open per mg (8 banks total incl.
double buffering). fc2 thus runs at the MAC-optimal PE cycle count:
nnz/4 * 512 cycles, ~1.6x fewer tensor cycles than a 128x128 2x2-group
formulation.

All tensors are stored/streamed as bf16 (same PE throughput as f32r,
half the HBM traffic; PSUM accumulation stays fp32).
"""

import numpy as np

import concourse.bass as bass
import concourse.bacc as bacc
import concourse.mybir as mybir
from concourse import tile
from concourse.bass_utils import run_bass_kernel_spmd

BS = 64  # BSR block size
N_CORES = 8

# matmul dtype mode: "f32" | "f32r" | "bf16"
MM_MODE = "bf16"
# fc2 scheme: "ladder" (64x64 quadrant open-shop) | "groups" (2x2 128x128)
SCHEME = "ladder"
CAP_OPEN = 3  # max concurrently-accumulating rows per mg (PSUM pressure)


def _np_dt(dt):
    return mybir.dt.np(dt)


def _mask_sig(mask):
    import hashlib

    return hashlib.sha256(np.packbits(mask.astype(bool)).tobytes()).hexdigest()[:16]


# =====================================================================
# Ladder schedule construction (host side)
# =====================================================================

# Precomputed for the canonical BSR mask this problem generates
# (np.random.default_rng(0), density 0.5, col 0 forced): col parity and
# row mg assignments with perfectly balanced lane loads (528 each).
_KNOWN_SIG = "25b40de11a15c565"
_KNOWN_PAR_HEX = "569d112bb4765ae8"  # jitter-scanned greedy reaches optimal 528
_KNOWN_MG_HEX = "ce939ef9dd0001ba"


def _bits_to_hex(bits):
    return np.packbits(np.asarray(bits, np.uint8)).tobytes().hex()


def _hex_to_bits(h, n=64):
    return np.unpackbits(np.frombuffer(bytes.fromhex(h), np.uint8))[:n].astype(np.int8)


def _lane_loads(mask, par, mg):
    L = np.stack([mask @ (par == 0), mask @ (par == 1)], 1).astype(np.int64)
    loads = np.zeros((2, 2), np.int64)
    for m in (0, 1):
        sel = mg == m
        loads[0, m] = L[sel, 0].sum()
        loads[1, m] = L[sel, 1].sum()
    return loads, L


def anneal_balance(mask, iters=80000, seed=0):
    """Choose col parity (32/32) and row mg (32/32) minimizing the max
    quadrant-lane load max_{kg,mg} sum_{r in mg} L[r,kg]."""
    rng = np.random.default_rng(seed)
    n = mask.shape[0]
    par = np.zeros(n, np.int8)
    par[rng.permutation(n)[: n // 2]] = 1
    mg = np.zeros(n, np.int8)
    mg[rng.permutation(n)[: n // 2]] = 1
    total = int(mask.sum())
    ideal = (total + 3) // 4

    def cost(par, mg):
        loads, _ = _lane_loads(mask, par, mg)
        return int(loads.max()) * 10000 + int((loads.astype(float) ** 2).sum() / 100)

    cur = cost(par, mg)
    best, best_state = cur, (par.copy(), mg.copy())
    for it in range(iters):
        T = 2000.0 * (1.0 / 2000.0) ** (it / max(1, iters - 1))
        which = rng.random() < 0.5
        if which:
            c0, c1 = rng.integers(n, size=2)
            if par[c0] == par[c1]:
                continue
            par[c0], par[c1] = par[c1], par[c0]
        else:
            r0, r1 = rng.integers(n, size=2)
            if mg[r0] == mg[r1]:
                continue
            mg[r0], mg[r1] = mg[r1], mg[r0]
        new = cost(par, mg)
        if new <= cur or rng.random() < np.exp((cur - new) / T):
            cur = new
            if new < best:
                best, best_state = new, (par.copy(), mg.copy())
                if best // 10000 <= ideal:
                    break
        else:
            if which:
                par[c0], par[c1] = par[c1], par[c0]
            else:
                mg[r0], mg[r1] = mg[r1], mg[r0]
    return best_state


GAP = 3       # min slots between a row's chain1 end and chain2 start
              # (hides the DVE sync read latency)
RING = 4      # PSUM bank ring depth per mg (2 tags x 4 bufs = 8 banks)
MERGE_SLOTS = 2  # slots after chain2 end until the bank is drained (ACT)


def greedy_open_shop(L, rows, cap=CAP_OPEN, gap=GAP, ring=RING, jitter_seed=None):
    """Schedule rows (jobs with ops L[r,0] on lane0 / L[r,1] on lane1,
    ops non-overlapping per row with >= gap slots between them, <= cap
    rows open, bank ring reuse distance `ring`) on 2 lanes.
    Returns (makespan, seq) with seq[k] = list of (start, row, ln)."""
    T = [0, 0]
    pending = []  # (ready_time, row, lane, ln) second ops
    unopened = list(rows)
    seq = {0: [], 1: []}
    n_done, n = 0, len(rows)
    open_order = []   # rows in bank-allocation order
    bank_free = {}    # row -> time its bank is drained
    if jitter_seed is None:
        jit = {r: 0.0 for r in rows}
    else:
        u = np.random.default_rng(jitter_seed).random(len(rows))
        jit = {r: float(u[i]) for i, r in enumerate(rows)}
    guard = 0
    while n_done < n:
        guard += 1
        assert guard < 100000, "greedy_open_shop failed to converge"
        k = 0 if T[0] <= T[1] else 1
        ko = 1 - k
        cands = sorted(p for p in pending if p[2] == k)
        can_open = bool(unopened) and len(pending) < cap
        if can_open and len(open_order) >= ring:
            prev = open_order[len(open_order) - ring]
            if bank_free.get(prev, None) is None or bank_free[prev] > T[k]:
                can_open = False
        did = False
        if cands:
            rt, r, _, ln = cands[0]
            if rt <= T[k] or not can_open:
                start = max(T[k], rt)
                if ln > 0:
                    seq[k].append((start, r, ln))
                    T[k] = start + ln
                    bank_free[r] = T[k] + MERGE_SLOTS
                else:
                    bank_free[r] = start + MERGE_SLOTS
                pending.remove((rt, r, k, ln))
                n_done += 1
                did = True
        if not did and can_open:
            unopened.sort(
                key=lambda r: -(int(L[r, k]) - int(L[r, ko])) - jit[r]
            )
            r = unopened.pop(0)
            ln = int(L[r, k])
            open_order.append(r)
            if ln > 0:
                seq[k].append((T[k], r, ln))
                T[k] += ln
                pending.append((T[k] + gap, r, ko, int(L[r, ko])))
            else:
                pending.append((T[k], r, ko, int(L[r, ko])))
            did = True
        if not did:
            nxt = min([p[0] for p in pending], default=T[ko])
            T[k] = max(T[k] + 1, nxt)
    return max(T), seq


def build_ladder(crow, col, nbr, cap=CAP_OPEN):
    """Build the full fc2 ladder schedule. Returns a dict with:
      S: number of slots
      lanes[(kg,mg)]: length-S list of None | (row, col, first, last)
      prow, pcol: position -> physical row/col permutations
      row_pos, col_pos: physical -> position
      mg_of_row, par_of_col
    """
    mask = np.zeros((nbr, nbr), np.int64)
    for br in range(nbr):
        for idx in range(int(crow[br]), int(crow[br + 1])):
            mask[br, int(col[idx])] = 1

    sig = _mask_sig(mask)
    if nbr == 64 and sig == _KNOWN_SIG and _KNOWN_PAR_HEX:
        par = _hex_to_bits(_KNOWN_PAR_HEX, nbr)
        mg = _hex_to_bits(_KNOWN_MG_HEX, nbr)
    else:
        # try a few anneal seeds, keep the one with best greedy makespan
        best = None
        for seed in range(4):
            p2, m2 = anneal_balance(mask, iters=60000, seed=seed)
            _, L2 = _lane_loads(mask, p2, m2)
            span = max(
                greedy_open_shop(
                    L2, [r for r in range(nbr) if m2[r] == m], cap=cap
                )[0]
                for m in (0, 1)
            )
            if best is None or span < best[0]:
                best = (span, p2, m2)
        par, mg = best[1], best[2]

    loads, L = _lane_loads(mask, par, mg)

    lanes = {}
    closure = {}
    first_lane = {}
    spans = []
    seqs = {}
    for m in (0, 1):
        rows = [r for r in range(nbr) if mg[r] == m]
        best = None
        for js in [None] + list(range(200)):
            span, seq = greedy_open_shop(L, rows, cap=cap, jitter_seed=js)
            if best is None or span < best[0]:
                best = (span, seq)
        spans.append(best[0])
        seqs[m] = best[1]
    S = max(spans)
    for m in (0, 1):
        seq = seqs[m]
        # determine op order per row (seq only contains nonzero ops)
        ops = {}
        for k in (0, 1):
            for start, r, ln in seq[k]:
                ops.setdefault(r, []).append((start, k, ln))
        for r, lst in ops.items():
            lst.sort()
            first_lane[r] = lst[0][1]
            closure[r] = lst[-1][0] + lst[-1][2]
        for k in (0, 1):
            lane = [None] * S
            for start, r, ln in seq[k]:
                cols = sorted(np.nonzero(mask[r] * (par == k))[0].tolist())
                assert len(cols) == ln
                is_first_op = (ops[r][0][1] == k) and (ops[r][0][0] == start)
                is_last_op = (ops[r][-1][1] == k) and (ops[r][-1][0] == start)
                for j in range(ln):
                    # (row, col, bass_start, bass_stop, sync_after)
                    lane[start + j] = (
                        r,
                        cols[j],
                        is_first_op and j == 0,
                        is_last_op and j == ln - 1,
                        (not is_last_op) and j == ln - 1,
                    )
            lanes[(k, m)] = lane

    # positions: mg0 rows -> even positions by closure order; mg1 -> odd
    prow = np.zeros(nbr, np.int64)
    row_pos = np.zeros(nbr, np.int64)
    for m in (0, 1):
        rows = [r for r in range(nbr) if mg[r] == m]
        rows.sort(key=lambda r: (closure[r], r))
        for i, r in enumerate(rows):
            p = 2 * i + m
            prow[p] = r
            row_pos[r] = p
    pcol = np.zeros(nbr, np.int64)
    col_pos = np.zeros(nbr, np.int64)
    for k in (0, 1):
        cols = [c for c in range(nbr) if par[c] == k]
        for i, c in enumerate(cols):
            q = 2 * i + k
            pcol[q] = c
            col_pos[c] = q

    # block index lookup
    bidx = {}
    for br in range(nbr):
        for idx in range(int(crow[br]), int(crow[br + 1])):
            bidx[(br, int(col[idx]))] = idx

    n_mm = sum(
        1 for ln in lanes.values() for e in ln if e is not None
    )
    assert n_mm == int(mask.sum()), (n_mm, int(mask.sum()))

    return {
        "scheme": "ladder",
        "S": S,
        "lanes": lanes,
        "prow": prow,
        "pcol": pcol,
        "row_pos": row_pos,
        "col_pos": col_pos,
        "mg": mg,
        "par": par,
        "bidx": bidx,
        "nbr": nbr,
    }


def pack_v2_ladder(values, sched, store_np):
    """Pack fc2 blocks into [128, S*128]: slot s holds the 4 quadrant
    blocks: (kg,mg) at [kg*64:(kg+1)*64, s*128+mg*64 : s*128+mg*64+64],
    laid out as lhsT (block.T)."""
    S = sched["S"]
    lanes = sched["lanes"]
    bidx = sched["bidx"]
    v2 = np.zeros((128, S * 128), np.float32)
    for (kg, mg), lane in lanes.items():
        for s, e in enumerate(lane):
            if e is None:
                continue
            r, c = e[0], e[1]
            v2[
                kg * 64 : (kg + 1) * 64,
                s * 128 + mg * 64 : s * 128 + mg * 64 + 64,
            ] = values[bidx[(r, c)]].T
    return np.ascontiguousarray(v2.astype(store_np))


# =====================================================================
# Legacy 2x2-group scheme (kept for A/B benchmarking)
# =====================================================================

_KNOWN_PR = [52, 37, 12, 42, 35, 11, 27, 50, 33, 17, 38, 30, 1, 40, 21, 26, 14, 44, 63, 19, 18, 59, 24, 60, 43, 55, 0, 54, 28, 7, 8, 22, 20, 25, 61, 13, 34, 32, 51, 57, 36, 49, 31, 47, 2, 15, 39, 41, 58, 9, 56, 6, 16, 45, 62, 5, 10, 48, 3, 53, 46, 29, 4, 23]
_KNOWN_PC = [6, 51, 49, 33, 8, 22, 1, 18, 13, 50, 21, 5, 15, 0, 2, 25, 52, 41, 38, 9, 7, 37, 4, 63, 3, 14, 20, 60, 62, 35, 61, 17, 57, 11, 39, 34, 19, 58, 46, 54, 23, 16, 42, 30, 28, 12, 36, 32, 24, 47, 43, 59, 53, 27, 26, 40, 55, 10, 29, 45, 44, 48, 31, 56]


def optimize_pairing(mask, iters=60000, rounds=4, seed=0):
    rng = np.random.default_rng(seed)
    nr, nc = mask.shape
    prow = list(range(nr))
    pcol = list(range(nc))

    def anneal(perm, bits, iters):
        n = len(perm)

        def paircost(i):
            return (bits[perm[2 * i]] | bits[perm[2 * i + 1]]).bit_count()

        cost = [paircost(i) for i in range(n // 2)]
        u = rng.random(iters)
        idx = rng.integers(0, n, (iters, 2))
        T0, T1 = 1.5, 0.02
        for it in range(iters):
            a, b = idx[it]
            ia, ib = a // 2, b // 2
            if ia == ib:
                continue
            perm[a], perm[b] = perm[b], perm[a]
            na, nb = paircost(ia), paircost(ib)
            d = na + nb - cost[ia] - cost[ib]
            T = T0 * (T1 / T0) ** (it / iters)
            if d <= 0 or u[it] < np.exp(-d / T):
                cost[ia], cost[ib] = na, nb
            else:
                perm[a], perm[b] = perm[b], perm[a]

    for _ in range(rounds):
        rowbits = [
            int.from_bytes(
                np.packbits(
                    (mask[r, pcol].reshape(nc // 2, 2).any(axis=1)), bitorder="little"
                ).tobytes(),
                "little",
            )
            for r in range(nr)
        ]
        anneal(prow, rowbits, iters)
        colbits = [
            int.from_bytes(
                np.packbits(
                    (mask[prow, c].reshape(nr // 2, 2).any(axis=1)), bitorder="little"
                ).tobytes(),
                "little",
            )
            for c in range(nc)
        ]
        anneal(pcol, colbits, iters)
    return np.array(prow), np.array(pcol)


def build_groups(crow, col, nbr):
    blocks = {}
    mask = np.zeros((nbr, nbr), bool)
    for br in range(nbr):
        for idx in range(int(crow[br]), int(crow[br + 1])):
            c = int(col[idx])
            blocks[(br, c)] = idx
            mask[br, c] = True
    if mask.shape == (64, 64) and _mask_sig(mask) == _KNOWN_SIG:
        prow, pcol = np.array(_KNOWN_PR), np.array(_KNOWN_PC)
    else:
        prow, pcol = optimize_pairing(mask)
    pblocks = {}
    for i in range(nbr):
        for j in range(nbr):
            idx = blocks.get((int(prow[i]), int(pcol[j])))
            if idx is not None:
                pblocks[(i, j)] = idx
    R2 = nbr // 2
    groups = []
    for r2 in range(R2):
        lst = []
        for t in range(R2):
            if any(
                (2 * r2 + ir, 2 * t + ic) in pblocks for ir in (0, 1) for ic in (0, 1)
            ):
                lst.append(t)
        groups.append(lst)
    return {
        "scheme": "groups",
        "groups": groups,
        "pblocks": pblocks,
        "prow": prow,
        "pcol": pcol,
        "nbr": nbr,
    }


def pack_v2_groups(values, sched, store_np):
    groups, blocks = sched["groups"], sched["pblocks"]
    G = sum(len(g) for g in groups)
    v2 = np.zeros((128, G * 128), np.float32)
    g = 0
    for r2, lst in enumerate(groups):
        for t in lst:
            Z = np.zeros((128, 128), np.float32)
            for ir in (0, 1):
                for ic in (0, 1):
                    idx = blocks.get((2 * r2 + ir, 2 * t + ic))
                    if idx is not None:
                        Z[ic * 64 : ic * 64 + 64, ir * 64 : ir * 64 + 64] = values[
                            idx
                        ].T
            v2[:, g * 128 : (g + 1) * 128] = Z
            g += 1
    return np.ascontiguousarray(v2.astype(store_np))


# =====================================================================
# Bass program
# =====================================================================


def build_nc(BSH, D_IN, H, D_OUT, sched, mode, repeat=1, quad=None, phases="ABC"):
    """Build the per-core Bass program (SPMD: same program on all cores)."""
    KI, MH, MO = D_IN // 128, H // 128, D_OUT // 128
    if sched["scheme"] == "ladder":
        G = sched["S"]
    else:
        G = sum(len(g) for g in sched["groups"])
    f32 = mybir.dt.float32
    if mode == "bf16":
        DT = mybir.dt.bfloat16
    elif mode == "f32r":
        DT = mybir.dt.float32r
    else:
        DT = f32

    nc = bacc.Bacc(None, target_bir_lowering=False)
    xp = nc.declare_dram_parameter("xp", [128, KI * BSH], DT, isOutput=False)
    w1p = nc.declare_dram_parameter("w1p", [128, MH * D_IN], DT, isOutput=False)
    b1p = nc.declare_dram_parameter("b1p", [128, MH], f32, isOutput=False)
    v2p = nc.declare_dram_parameter("v2p", [128, G * 128], DT, isOutput=False)
    w3p = nc.declare_dram_parameter("w3p", [128, MO * H], DT, isOutput=False)
    b3p = nc.declare_dram_parameter("b3p", [128, MO], f32, isOutput=False)
    yp = nc.declare_dram_parameter("yp", [128, MO * BSH], f32, isOutput=True)

    with tile.TileContext(nc) as tc:
        for _rep in range(repeat):
            _build_body(
                nc, tc, xp, w1p, b1p, v2p, w3p, b3p, yp, BSH, D_IN, H, D_OUT,
                sched, DT, phases=phases,
            )
    nc.compile()
    return nc


def _fc2_ladder(nc, tc, v2p, h_tiles, h2_tiles, sched, BSH, DT):
    """Emit the ladder-scheme fc2: 4 concurrent 64x64 quadrant lanes,
    per-row half-bank accumulation, ACT relu on retire."""
    f32 = mybir.dt.float32
    Relu = mybir.ActivationFunctionType.Relu
    S = sched["S"]
    lanes = sched["lanes"]
    row_pos = sched["row_pos"]
    col_pos = sched["col_pos"]
    STRIP = 48
    row_psum = {}
    with (
        tc.tile_pool(name="v2pool", bufs=3) as v2pool,
        tc.tile_pool(name="scratch", bufs=1) as scpool,
        tc.tile_pool(name="qpsum", bufs=4, space="PSUM") as qpool,
    ):
        sct = scpool.tile([128, 8], f32, name="sct")
        for s0 in range(0, S, STRIP):
            w = min(STRIP, S - s0)
            vt = v2pool.tile([128, w * 128], DT, tag="v2")
            nc.sync.dma_start(out=vt[:], in_=v2p[:, s0 * 128 : (s0 + w) * 128])
            for s in range(s0, s0 + w):
                for kg, mg in ((0, 0), (1, 0), (0, 1), (1, 1)):
                    e = lanes[(kg, mg)][s]
                    if e is None:
                        continue
                    r, c, bstart, bstop, sync_after = e
                    if bstart:
                        row_psum[r] = qpool.tile(
                            [128, BSH], f32, tag=f"q{mg}", name=f"q_{r}"
                        )
                    ps = row_psum[r]
                    q = int(col_pos[c])
                    off = (s - s0) * 128 + mg * 64
                    nc.tensor.matmul(
                        ps[mg * 64 : (mg + 1) * 64, :],
                        lhsT=vt[kg * 64 : (kg + 1) * 64, off : off + 64],
                        rhs=h_tiles[q // 2][kg * 64 : (kg + 1) * 64, :],
                        start=bstart,
                        stop=bstop,
                        tile_position=(kg * 64, mg * 64),
                        skip_group_check=True,
                    )
                    if sync_after:
                        # Completion fence between this row's two chains:
                        # DVE reads the bank (waits chain1 completion);
                        # chain2's first MM gets a WAR dep on this read,
                        # so the two chains can never overlap in time
                        # (same PSUM bank from different PE row-groups
                        # would be a fatal collision).
                        nc.vector.tensor_copy(
                            sct[mg * 64 : mg * 64 + 64, :],
                            ps[mg * 64 : (mg + 1) * 64, 0:8],
                        )
                    if bstop:
                        p = int(row_pos[r])
                        assert p % 2 == mg
                        nc.scalar.activation(
                            h2_tiles[p // 2][mg * 64 : (mg + 1) * 64, :],
                            ps[mg * 64 : (mg + 1) * 64, :],
                            Relu,
                        )


def _build_body(nc, tc, xp, w1p, b1p, v2p, w3p, b3p, yp, BSH, D_IN, H, D_OUT, sched, DT, phases="ABC"):
    KI, MH, MO = D_IN // 128, H // 128, D_OUT // 128
    f32 = mybir.dt.float32
    Relu = mybir.ActivationFunctionType.Relu
    Ident = mybir.ActivationFunctionType.Identity
    with (
        tc.tile_pool(name="consts", bufs=1) as constp,
        tc.tile_pool(name="h2pool", bufs=1) as h2pool,
    ):
        b1t = constp.tile([128, MH], f32)
        nc.sync.dma_start(out=b1t[:], in_=b1p[:, :])
        b3t = constp.tile([128, MO], f32)
        nc.sync.dma_start(out=b3t[:], in_=b3p[:, :])

        with tc.tile_pool(name="hpool", bufs=1) as hpool:
            h_tiles = []
            # ---- Phase A: hT = relu(W1 @ xT + b1) ----
            with (
                tc.tile_pool(name="xpool", bufs=1) as xpool,
                tc.tile_pool(name="w1pool", bufs=4) as w1pool,
                tc.tile_pool(name="psumA", bufs=4, space="PSUM") as psumA,
            ):
                xt = xpool.tile([128, KI * BSH], DT)
                nc.sync.dma_start(out=xt[:], in_=xp[:, :])
                for mt in range(MH):
                    wt = w1pool.tile([128, D_IN], DT, tag="w1")
                    nc.sync.dma_start(
                        out=wt[:], in_=w1p[:, mt * D_IN : (mt + 1) * D_IN]
                    )
                    ps = psumA.tile([128, BSH], f32, tag="ps")
                    for n in range(KI):
                        nc.tensor.matmul(
                            ps[:],
                            lhsT=wt[:, n * 128 : (n + 1) * 128],
                            rhs=xt[:, n * BSH : (n + 1) * BSH],
                            start=(n == 0),
                            stop=(n == KI - 1),
                        )
                    ht = hpool.tile([128, BSH], DT, tag=f"h{mt}")
                    nc.scalar.activation(
                        ht[:], ps[:], Relu, bias=b1t[:, mt : mt + 1]
                    )
                    h_tiles.append(ht)

            if "B" not in phases:
                # timing probe: flush last h tile so phase A isn't dead
                nc.sync.dma_start(
                    out=yp[:, 0 : BSH // 2], in_=h_tiles[-1][:].bitcast(f32)
                )
                return
            # ---- Phase B: h2T = relu(W2_bsr @ hT) ----
            h2_tiles = [
                h2pool.tile([128, BSH], DT, tag=f"h2_{i}", name=f"h2_{i}")
                for i in range(MH)
            ]
            if sched["scheme"] == "ladder":
                _fc2_ladder(nc, tc, v2p, h_tiles, h2_tiles, sched, BSH, DT)
            else:
                groups = sched["groups"]
                with (
                    tc.tile_pool(name="v2pool", bufs=3) as v2pool,
                    tc.tile_pool(name="psumB", bufs=4, space="PSUM") as psumB,
                ):
                    g0 = 0
                    for r2, lst in enumerate(groups):
                        ng = len(lst)
                        vt = v2pool.tile([128, ng * 128], DT, tag="v2")
                        nc.sync.dma_start(
                            out=vt[:], in_=v2p[:, g0 * 128 : (g0 + ng) * 128]
                        )
                        ps = psumB.tile([128, BSH], f32, tag="ps")
                        for j, t in enumerate(lst):
                            nc.tensor.matmul(
                                ps[:],
                                lhsT=vt[:, j * 128 : (j + 1) * 128],
                                rhs=h_tiles[t][:],
                                start=(j == 0),
                                stop=(j == ng - 1),
                            )
                        nc.scalar.activation(h2_tiles[r2][:], ps[:], Relu)
                        g0 += ng

        if "C" not in phases:
            nc.sync.dma_start(
                out=yp[:, 0 : BSH // 2], in_=h2_tiles[-1][:].bitcast(f32)
            )
            return
        # ---- Phase C: yT = W3 @ h2T + b3 ----
        with (
            tc.tile_pool(name="w3pool", bufs=3) as w3pool,
            tc.tile_pool(name="ypool", bufs=1) as ypool,
            tc.tile_pool(name="psumC", bufs=4, space="PSUM") as psumC,
        ):
            yt = ypool.tile([128, MO * BSH], f32)
            for mo in range(MO):
                wt = w3pool.tile([128, H], DT, tag="w3")
                nc.sync.dma_start(out=wt[:], in_=w3p[:, mo * H : (mo + 1) * H])
                ps = psumC.tile([128, BSH], f32, tag="ps")
                for k in range(MH):
                    nc.tensor.matmul(
                        ps[:],
                        lhsT=wt[:, k * 128 : (k + 1) * 128],
                        rhs=h2_tiles[k][:],
                        start=(k == 0),
                        stop=(k == MH - 1),
                    )
                nc.scalar.activation(
                    yt[:, mo * BSH : (mo + 1) * BSH],
                    ps[:],
                    Ident,
                    bias=b3t[:, mo : mo + 1],
                )
            nc.sync.dma_start(out=yp[:, :], in_=yt[:])


# =====================================================================
# Host packing / run
# =====================================================================


def pack_inputs(
    x, w1, b1, values, w3, b3, crow, col, mode, n_cores=N_CORES, scheme=SCHEME,
    use_quad=False,
):
    """Host-side swizzle of all tensors into the DRAM layouts build_nc
    expects. Returns (shared_map, per_core_xp, sched, None)."""
    B, D_IN = x.shape
    H = w1.shape[0]
    D_OUT = w3.shape[0]
    KI, MH, MO = D_IN // 128, H // 128, D_OUT // 128
    BSH = B // n_cores
    store_np = _np_dt(mybir.dt.bfloat16) if mode == "bf16" else np.float32

    nbr = H // BS
    if scheme == "ladder":
        sched = build_ladder(crow, col, nbr)
        v2p = pack_v2_ladder(values, sched, store_np)
    else:
        sched = build_groups(crow, col, nbr)
        v2p = pack_v2_groups(values, sched, store_np)
    prow, pcol = sched["prow"], sched["pcol"]

    # fc1 output rows (= fc2 input block-cols) permuted by pcol;
    # fc3 contraction cols (= fc2 output block-rows) permuted by prow.
    w1 = w1.reshape(nbr, BS, D_IN)[pcol].reshape(H, D_IN)
    b1 = b1.reshape(nbr, BS)[pcol].reshape(H)
    w3 = w3.reshape(D_OUT, nbr, BS)[:, prow].reshape(D_OUT, H)

    w1p = np.ascontiguousarray(
        w1.reshape(MH, 128, KI, 128).transpose(3, 0, 2, 1).reshape(128, MH * D_IN)
    ).astype(store_np)
    w3p = np.ascontiguousarray(
        w3.reshape(MO, 128, MH, 128).transpose(3, 0, 2, 1).reshape(128, MO * H)
    ).astype(store_np)
    b1p = np.ascontiguousarray(b1.reshape(MH, 128).T).astype(np.float32)
    b3p = np.ascontiguousarray(b3.reshape(MO, 128).T).astype(np.float32)

    shared = {"w1p": w1p, "b1p": b1p, "v2p": v2p, "w3p": w3p, "b3p": b3p}
    xps = []
    for c in range(n_cores):
        xs = x[c * BSH : (c + 1) * BSH]
        xps.append(
            np.ascontiguousarray(
                xs.reshape(BSH, KI, 128).transpose(2, 1, 0).reshape(128, KI * BSH)
            ).astype(store_np)
        )
    return shared, xps, sched, None


def unpack_output(yps, B, D_OUT, n_cores=N_CORES):
    BSH = B // n_cores
    MO = D_OUT // 128
    out = np.empty((B, D_OUT), np.float32)
    for c, yp in enumerate(yps):
        out[c * BSH : (c + 1) * BSH] = (
            yp.reshape(128, MO, BSH).transpose(2, 1, 0).reshape(BSH, MO * 128)
        )
    return out


def run(x, w1, b1, values, w3, b3, crow, col, mode=MM_MODE, scheme=SCHEME, trace=False):
    B, D_IN = x.shape
    H = w1.shape[0]
    D_OUT = w3.shape[0]
    BSH = B // N_CORES
    shared, xps, sched, _ = pack_inputs(
        x, w1, b1, values, w3, b3, crow, col, mode, scheme=scheme
    )
    nc = build_nc(BSH, D_IN, H, D_OUT, sched, mode)
    in_maps = [dict(shared, xp=xps[c]) for c in range(N_CORES)]
    res = run_bass_kernel_spmd(nc, in_maps, core_ids=list(range(N_CORES)), trace=trace)
    out = unpack_output([res.results[c]["yp"] for c in range(N_CORES)], B, D_OUT)
    return out, res


def kernel(x, w1, b1, values, w3, b3, crow_indices, col_indices):
    x = np.asarray(x, np.float32)
    w1 = np.asarray(w1, np.float32)
    b1 = np.asarray(b1, np.float32)
    values = np.asarray(values, np.float32)
    w3 = np.asarray(w3, np.float32)
    b3 = np.asarray(b3, np.float32)
    crow = np.asarray(crow_indices)
    col = np.asarray(col_indices)
    out, _ = run(x, w1, b1, values, w3, b3, crow, col)
    return out



# revision 7
# speedup vs baseline: 1.0597x; 1.0597x over previous
"""Trainium2 Bass kernel for BlockedMLP:
    h1 = relu(x @ w1.T + b1)            # dense fc1
    h2 = relu(bsr_linear(h1, W2_bsr))   # 64x64-blocked sparse fc2
    y  = h2 @ w3.T + b3                 # dense fc3

Strategy: data-parallel over the batch dim across 8 NeuronCores
(weights replicated, no collectives). Everything is computed in a
feature-major ("transposed") layout so matmuls contract over the
partition dim with N = Bsh = 512 batch columns:

    hT  [H, Bsh]   = W1 @ xT      (Bsh = 4096/8 = 512 batch rows/core)
    h2T [H, Bsh]   = W2 @ hT      (BSR: compile-time-known sparsity)
    yT  [Dout,Bsh] = W3 @ h2T

fc1/fc3 are dense 128x128xK matmul chains at full PE rate. fc2 uses the
"ladder" scheme: the PE array is split into 4 concurrent 64x64 tiles
via tile_position. Quadrant (kg, mg) = (rhs partition half, psum/out
partition half). Each of the 64 BSR block rows is assigned an output
half mg and accumulates ALL its blocks into one half PSUM bank via two
sequential chains (cols with parity kg=0, then kg=1 — order can differ
per row), so no cross-bank merge is needed. A host-side balance anneal
picks the col parity classes + row mg split so all 4 quadrant lanes
carry exactly nnz/4 blocks, and a bounded-open-rows greedy open-shop
schedule achieves the optimal makespan (528 slots for the canonical
mask) with at most 3 rows' PSUM banks open per mg (8 banks total incl.
double buffering). fc2 thus runs at the MAC-optimal PE cycle count:
nnz/4 * 512 cycles, ~1.6x fewer tensor cycles than a 128x128 2x2-group
formulation.

All tensors are stored/streamed as bf16 (same PE throughput as f32r,
half the HBM traffic; PSUM accumulation stays fp32).
"""

import numpy as np

import concourse.bass as bass
import concourse.bacc as bacc
import concourse.mybir as mybir
from concourse import tile
from concourse.bass_utils import run_bass_kernel_spmd

BS = 64  # BSR block size
N_CORES = 8

# matmul dtype mode: "f32" | "f32r" | "bf16"
MM_MODE = "bf16"
# fc2 scheme: "ladder" (64x64 quadrant open-shop) | "groups" (2x2 128x128)
SCHEME = "ladder"
CAP_OPEN = 3  # max concurrently-accumulating rows per mg (PSUM pressure)


def _np_dt(dt):
    return mybir.dt.np(dt)


def _mask_sig(mask):
    import hashlib

    return hashlib.sha256(np.packbits(mask.astype(bool)).tobytes()).hexdigest()[:16]


# =====================================================================
# Ladder schedule construction (host side)
# =====================================================================

# Precomputed for the canonical BSR mask this problem generates
# (np.random.default_rng(0), density 0.5, col 0 forced): col parity and
# row mg assignments with perfectly balanced lane loads (528 each).
_KNOWN_SIG = "25b40de11a15c565"
_KNOWN_PAR_HEX = "569d112bb4765ae8"  # jitter-scanned greedy reaches optimal 528
_KNOWN_MG_HEX = "ce939ef9dd0001ba"


def _bits_to_hex(bits):
    return np.packbits(np.asarray(bits, np.uint8)).tobytes().hex()


def _hex_to_bits(h, n=64):
    return np.unpackbits(np.frombuffer(bytes.fromhex(h), np.uint8))[:n].astype(np.int8)


def _lane_loads(mask, par, mg):
    L = np.stack([mask @ (par == 0), mask @ (par == 1)], 1).astype(np.int64)
    loads = np.zeros((2, 2), np.int64)
    for m in (0, 1):
        sel = mg == m
        loads[0, m] = L[sel, 0].sum()
        loads[1, m] = L[sel, 1].sum()
    return loads, L


def anneal_balance(mask, iters=80000, seed=0):
    """Choose col parity (32/32) and row mg (32/32) minimizing the max
    quadrant-lane load max_{kg,mg} sum_{r in mg} L[r,kg]."""
    rng = np.random.default_rng(seed)
    n = mask.shape[0]
    par = np.zeros(n, np.int8)
    par[rng.permutation(n)[: n // 2]] = 1
    mg = np.zeros(n, np.int8)
    mg[rng.permutation(n)[: n // 2]] = 1
    total = int(mask.sum())
    ideal = (total + 3) // 4

    def cost(par, mg):
        loads, _ = _lane_loads(mask, par, mg)
        return int(loads.max()) * 10000 + int((loads.astype(float) ** 2).sum() / 100)

    cur = cost(par, mg)
    best, best_state = cur, (par.copy(), mg.copy())
    for it in range(iters):
        T = 2000.0 * (1.0 / 2000.0) ** (it / max(1, iters - 1))
        which = rng.random() < 0.5
        if which:
            c0, c1 = rng.integers(n, size=2)
            if par[c0] == par[c1]:
                continue
            par[c0], par[c1] = par[c1], par[c0]
        else:
            r0, r1 = rng.integers(n, size=2)
            if mg[r0] == mg[r1]:
                continue
            mg[r0], mg[r1] = mg[r1], mg[r0]
        new = cost(par, mg)
        if new <= cur or rng.random() < np.exp((cur - new) / T):
            cur = new
            if new < best:
                best, best_state = new, (par.copy(), mg.copy())
                if best // 10000 <= ideal:
                    break
        else:
            if which:
                par[c0], par[c1] = par[c1], par[c0]
            else:
                mg[r0], mg[r1] = mg[r1], mg[r0]
    return best_state


GAP = 3       # min slots between a row's chain1 end and chain2 start
              # (hides the DVE sync read latency)
RING = 4      # PSUM bank ring depth per mg (2 tags x 4 bufs = 8 banks)
MERGE_SLOTS = 2  # slots after chain2 end until the bank is drained (ACT)


def greedy_open_shop(L, rows, cap=CAP_OPEN, gap=GAP, ring=RING, jitter_seed=None):
    """Schedule rows (jobs with ops L[r,0] on lane0 / L[r,1] on lane1,
    ops non-overlapping per row with >= gap slots between them, <= cap
    rows open, bank ring reuse distance `ring`) on 2 lanes.
    Returns (makespan, seq) with seq[k] = list of (start, row, ln)."""
    T = [0, 0]
    pending = []  # (ready_time, row, lane, ln) second ops
    unopened = list(rows)
    seq = {0: [], 1: []}
    n_done, n = 0, len(rows)
    open_order = []   # rows in bank-allocation order
    bank_free = {}    # row -> time its bank is drained
    if jitter_seed is None:
        jit = {r: 0.0 for r in rows}
    else:
        u = np.random.default_rng(jitter_seed).random(len(rows))
        jit = {r: float(u[i]) for i, r in enumerate(rows)}
    guard = 0
    while n_done < n:
        guard += 1
        assert guard < 100000, "greedy_open_shop failed to converge"
        k = 0 if T[0] <= T[1] else 1
        ko = 1 - k
        cands = sorted(p for p in pending if p[2] == k)
        can_open = bool(unopened) and len(pending) < cap
        if can_open and len(open_order) >= ring:
            prev = open_order[len(open_order) - ring]
            if bank_free.get(prev, None) is None or bank_free[prev] > T[k]:
                can_open = False
        did = False
        if cands:
            rt, r, _, ln = cands[0]
            if rt <= T[k] or not can_open:
                start = max(T[k], rt)
                if ln > 0:
                    seq[k].append((start, r, ln))
                    T[k] = start + ln
                    bank_free[r] = T[k] + MERGE_SLOTS
                else:
                    bank_free[r] = start + MERGE_SLOTS
                pending.remove((rt, r, k, ln))
                n_done += 1
                did = True
        if not did and can_open:
            unopened.sort(
                key=lambda r: -(int(L[r, k]) - int(L[r, ko])) - jit[r]
            )
            r = unopened.pop(0)
            ln = int(L[r, k])
            open_order.append(r)
            if ln > 0:
                seq[k].append((T[k], r, ln))
                T[k] += ln
                pending.append((T[k] + gap, r, ko, int(L[r, ko])))
            else:
                pending.append((T[k], r, ko, int(L[r, ko])))
            did = True
        if not did:
            nxt = min([p[0] for p in pending], default=T[ko])
            T[k] = max(T[k] + 1, nxt)
    return max(T), seq


def build_ladder(crow, col, nbr, cap=CAP_OPEN):
    """Build the full fc2 ladder schedule. Returns a dict with:
      S: number of slots
      lanes[(kg,mg)]: length-S list of None | (row, col, first, last)
      prow, pcol: position -> physical row/col permutations
      row_pos, col_pos: physical -> position
      mg_of_row, par_of_col
    """
    mask = np.zeros((nbr, nbr), np.int64)
    for br in range(nbr):
        for idx in range(int(crow[br]), int(crow[br + 1])):
            mask[br, int(col[idx])] = 1

    sig = _mask_sig(mask)
    if nbr == 64 and sig == _KNOWN_SIG and _KNOWN_PAR_HEX:
        par = _hex_to_bits(_KNOWN_PAR_HEX, nbr)
        mg = _hex_to_bits(_KNOWN_MG_HEX, nbr)
    else:
        # try a few anneal seeds, keep the one with best greedy makespan
        best = None
        for seed in range(4):
            p2, m2 = anneal_balance(mask, iters=60000, seed=seed)
            _, L2 = _lane_loads(mask, p2, m2)
            span = max(
                greedy_open_shop(
                    L2, [r for r in range(nbr) if m2[r] == m], cap=cap
                )[0]
                for m in (0, 1)
            )
            if best is None or span < best[0]:
                best = (span, p2, m2)
        par, mg = best[1], best[2]

    loads, L = _lane_loads(mask, par, mg)

    lanes = {}
    closure = {}
    first_lane = {}
    spans = []
    seqs = {}
    for m in (0, 1):
        rows = [r for r in range(nbr) if mg[r] == m]
        best = None
        for js in [None] + list(range(200)):
            span, seq = greedy_open_shop(L, rows, cap=cap, jitter_seed=js)
            if best is None or span < best[0]:
                best = (span, seq)
        spans.append(best[0])
        seqs[m] = best[1]
    S = max(spans)
    for m in (0, 1):
        seq = seqs[m]
        # determine op order per row (seq only contains nonzero ops)
        ops = {}
        for k in (0, 1):
            for start, r, ln in seq[k]:
                ops.setdefault(r, []).append((start, k, ln))
        for r, lst in ops.items():
            lst.sort()
            first_lane[r] = lst[0][1]
            closure[r] = lst[-1][0] + lst[-1][2]
        for k in (0, 1):
            lane = [None] * S
            for start, r, ln in seq[k]:
                cols = sorted(np.nonzero(mask[r] * (par == k))[0].tolist())
                assert len(cols) == ln
                is_first_op = (ops[r][0][1] == k) and (ops[r][0][0] == start)
                is_last_op = (ops[r][-1][1] == k) and (ops[r][-1][0] == start)
                for j in range(ln):
                    # (row, col, bass_start, bass_stop, sync_after)
                    lane[start + j] = (
                        r,
                        cols[j],
                        is_first_op and j == 0,
                        is_last_op and j == ln - 1,
                        (not is_last_op) and j == ln - 1,
                    )
            lanes[(k, m)] = lane

    # positions: mg0 rows -> even positions by closure order; mg1 -> odd
    prow = np.zeros(nbr, np.int64)
    row_pos = np.zeros(nbr, np.int64)
    for m in (0, 1):
        rows = [r for r in range(nbr) if mg[r] == m]
        rows.sort(key=lambda r: (closure[r], r))
        for i, r in enumerate(rows):
            p = 2 * i + m
            prow[p] = r
            row_pos[r] = p
    pcol = np.zeros(nbr, np.int64)
    col_pos = np.zeros(nbr, np.int64)
    for k in (0, 1):
        cols = [c for c in range(nbr) if par[c] == k]
        for i, c in enumerate(cols):
            q = 2 * i + k
            pcol[q] = c
            col_pos[c] = q

    # block index lookup
    bidx = {}
    for br in range(nbr):
        for idx in range(int(crow[br]), int(crow[br + 1])):
            bidx[(br, int(col[idx]))] = idx

    n_mm = sum(
        1 for ln in lanes.values() for e in ln if e is not None
    )
    assert n_mm == int(mask.sum()), (n_mm, int(mask.sum()))

    return {
        "scheme": "ladder",
        "S": S,
        "lanes": lanes,
        "prow": prow,
        "pcol": pcol,
        "row_pos": row_pos,
        "col_pos": col_pos,
        "mg": mg,
        "par": par,
        "bidx": bidx,
        "nbr": nbr,
    }


def pack_v2_ladder(values, sched, store_np):
    """Pack fc2 blocks into [128, S*128]: slot s holds the 4 quadrant
    blocks: (kg,mg) at [kg*64:(kg+1)*64, s*128+mg*64 : s*128+mg*64+64],
    laid out as lhsT (block.T)."""
    S = sched["S"]
    lanes = sched["lanes"]
    bidx = sched["bidx"]
    v2 = np.zeros((128, S * 128), np.float32)
    for (kg, mg), lane in lanes.items():
        for s, e in enumerate(lane):
            if e is None:
                continue
            r, c = e[0], e[1]
            v2[
                kg * 64 : (kg + 1) * 64,
                s * 128 + mg * 64 : s * 128 + mg * 64 + 64,
            ] = values[bidx[(r, c)]].T
    return np.ascontiguousarray(v2.astype(store_np))


# =====================================================================
# Legacy 2x2-group scheme (kept for A/B benchmarking)
# =====================================================================

_KNOWN_PR = [52, 37, 12, 42, 35, 11, 27, 50, 33, 17, 38, 30, 1, 40, 21, 26, 14, 44, 63, 19, 18, 59, 24, 60, 43, 55, 0, 54, 28, 7, 8, 22, 20, 25, 61, 13, 34, 32, 51, 57, 36, 49, 31, 47, 2, 15, 39, 41, 58, 9, 56, 6, 16, 45, 62, 5, 10, 48, 3, 53, 46, 29, 4, 23]
_KNOWN_PC = [6, 51, 49, 33, 8, 22, 1, 18, 13, 50, 21, 5, 15, 0, 2, 25, 52, 41, 38, 9, 7, 37, 4, 63, 3, 14, 20, 60, 62, 35, 61, 17, 57, 11, 39, 34, 19, 58, 46, 54, 23, 16, 42, 30, 28, 12, 36, 32, 24, 47, 43, 59, 53, 27, 26, 40, 55, 10, 29, 45, 44, 48, 31, 56]


def optimize_pairing(mask, iters=60000, rounds=4, seed=0):
    rng = np.random.default_rng(seed)
    nr, nc = mask.shape
    prow = list(range(nr))
    pcol = list(range(nc))

    def anneal(perm, bits, iters):
        n = len(perm)

        def paircost(i):
            return (bits[perm[2 * i]] | bits[perm[2 * i + 1]]).bit_count()

        cost = [paircost(i) for i in range(n // 2)]
        u = rng.random(iters)
        idx = rng.integers(0, n, (iters, 2))
        T0, T1 = 1.5, 0.02
        for it in range(iters):
            a, b = idx[it]
            ia, ib = a // 2, b // 2
            if ia == ib:
                continue
            perm[a], perm[b] = perm[b], perm[a]
            na, nb = paircost(ia), paircost(ib)
            d = na + nb - cost[ia] - cost[ib]
            T = T0 * (T1 / T0) ** (it / iters)
            if d <= 0 or u[it] < np.exp(-d / T):
                cost[ia], cost[ib] = na, nb
            else:
                perm[a], perm[b] = perm[b], perm[a]

    for _ in range(rounds):
        rowbits = [
            int.from_bytes(
                np.packbits(
                    (mask[r, pcol].reshape(nc // 2, 2).any(axis=1)), bitorder="little"
                ).tobytes(),
                "little",
            )
            for r in range(nr)
        ]
        anneal(prow, rowbits, iters)
        colbits = [
            int.from_bytes(
                np.packbits(
                    (mask[prow, c].reshape(nr // 2, 2).any(axis=1)), bitorder="little"
                ).tobytes(),
                "little",
            )
            for c in range(nc)
        ]
        anneal(pcol, colbits, iters)
    return np.array(prow), np.array(pcol)


def build_groups(crow, col, nbr):
    blocks = {}
    mask = np.zeros((nbr, nbr), bool)
    for br in range(nbr):
        for idx in range(int(crow[br]), int(crow[br + 1])):
            c = int(col[idx])
            blocks[(br, c)] = idx
            mask[br, c] = True
    if mask.shape == (64, 64) and _mask_sig(mask) == _KNOWN_SIG:
        prow, pcol = np.array(_KNOWN_PR), np.array(_KNOWN_PC)
    else:
        prow, pcol = optimize_pairing(mask)
    pblocks = {}
    for i in range(nbr):
        for j in range(nbr):
            idx = blocks.get((int(prow[i]), int(pcol[j])))
            if idx is not None:
                pblocks[(i, j)] = idx
    R2 = nbr // 2
    groups = []
    for r2 in range(R2):
        lst = []
        for t in range(R2):
            if any(
                (2 * r2 + ir, 2 * t + ic) in pblocks for ir in (0, 1) for ic in (0, 1)
            ):
                lst.append(t)
        groups.append(lst)
    return {
        "scheme": "groups",
        "groups": groups,
        "pblocks": pblocks,
        "prow": prow,
        "pcol": pcol,
        "nbr": nbr,
    }


def pack_v2_groups(values, sched, store_np):
    groups, blocks = sched["groups"], sched["pblocks"]
    G = sum(len(g) for g in groups)
    v2 = np.zeros((128, G * 128), np.float32)
    g = 0
    for r2, lst in enumerate(groups):
        for t in lst:
            Z = np.zeros((128, 128), np.float32)
            for ir in (0, 1):
                for ic in (0, 1):
                    idx = blocks.get((2 * r2 + ir, 2 * t + ic))
                    if idx is not None:
                        Z[ic * 64 : ic * 64 + 64, ir * 64 : ir * 64 + 64] = values[
                            idx
                        ].T
            v2[:, g * 128 : (g + 1) * 128] = Z
            g += 1
    return np.ascontiguousarray(v2.astype(store_np))


# =====================================================================
# Bass program
# =====================================================================


def build_nc(BSH, D_IN, H, D_OUT, sched, mode, repeat=1, quad=None, phases="ABC",
             warmup=True):
    """Build the per-core Bass program (SPMD: same program on all cores)."""
    KI, MH, MO = D_IN // 128, H // 128, D_OUT // 128
    if sched["scheme"] == "ladder":
        G = sched["S"]
    else:
        G = sum(len(g) for g in sched["groups"])
    f32 = mybir.dt.float32
    if mode == "bf16":
        DT = mybir.dt.bfloat16
    elif mode == "f32r":
        DT = mybir.dt.float32r
    else:
        DT = f32

    nc = bacc.Bacc(None, target_bir_lowering=False)
    xp = nc.declare_dram_parameter("xp", [128, KI * BSH], DT, isOutput=False)
    w1p = nc.declare_dram_parameter("w1p", [128, MH * D_IN], DT, isOutput=False)
    b1p = nc.declare_dram_parameter("b1p", [128, MH], f32, isOutput=False)
    v2p = nc.declare_dram_parameter("v2p", [128, G * 128], DT, isOutput=False)
    w3p = nc.declare_dram_parameter("w3p", [128, MO * H], DT, isOutput=False)
    b3p = nc.declare_dram_parameter("b3p", [128, MO], f32, isOutput=False)
    yp = nc.declare_dram_parameter("yp", [128, MO * BSH], f32, isOutput=True)

    with tile.TileContext(nc) as tc:
        if warmup and DT != f32:
            # One-time PE warm-up: ~8 dependency-free matmuls on a zeroed
            # tile keep TensorE busy through the HAM activity window while
            # the first input DMAs are in flight, so the real fc1 matmuls
            # start at 2.4 GHz instead of 1.2 GHz. Outside the repeat loop
            # (cost amortizes to ~0 in steady state).
            with (
                tc.tile_pool(name="warm", bufs=1) as wp,
                tc.tile_pool(name="warmps", bufs=1, space="PSUM") as wpp,
            ):
                zt = wp.tile([128, BSH], DT, name="warmz")
                nc.vector.memset(zt[:], 0.0)
                wps = wpp.tile([128, BSH], f32, name="warmp")
                for _ in range(8):
                    nc.tensor.matmul(
                        wps[:], lhsT=zt[:, 0:128], rhs=zt[:], start=True, stop=True
                    )
        for _rep in range(repeat):
            _build_body(
                nc, tc, xp, w1p, b1p, v2p, w3p, b3p, yp, BSH, D_IN, H, D_OUT,
                sched, DT, phases=phases,
            )
    nc.compile()
    return nc


FC2_STRIP = 48  # fc2 v2 DMA strip width (slots)


def _fc2_ladder(nc, tc, v2p, h_tiles, h2_tiles, sched, BSH, DT, v2_tiles, fetch_strip):
    """Emit the ladder-scheme fc2: 4 concurrent 64x64 quadrant lanes,
    per-row half-bank accumulation, ACT relu on retire. v2_tiles holds
    strips already prefetched during phase A; fetch_strip(si) DMAs the
    rest as their ring slots free up."""
    f32 = mybir.dt.float32
    Relu = mybir.ActivationFunctionType.Relu
    S = sched["S"]
    lanes = sched["lanes"]
    row_pos = sched["row_pos"]
    col_pos = sched["col_pos"]
    STRIP = FC2_STRIP
    row_psum = {}
    with (
        tc.tile_pool(name="scratch", bufs=1) as scpool,
        tc.tile_pool(name="qpsum", bufs=4, space="PSUM") as qpool,
    ):
        sct = scpool.tile([128, 8], f32, name="sct")
        for s0 in range(0, S, STRIP):
            si = s0 // STRIP
            w = min(STRIP, S - s0)
            if si not in v2_tiles:
                fetch_strip(si)
            vt = v2_tiles.pop(si)
            for s in range(s0, s0 + w):
                for kg, mg in ((0, 0), (1, 0), (0, 1), (1, 1)):
                    e = lanes[(kg, mg)][s]
                    if e is None:
                        continue
                    r, c, bstart, bstop, sync_after = e
                    if bstart:
                        row_psum[r] = qpool.tile(
                            [128, BSH], f32, tag=f"q{mg}", name=f"q_{r}"
                        )
                    ps = row_psum[r]
                    q = int(col_pos[c])
                    off = (s - s0) * 128 + mg * 64
                    nc.tensor.matmul(
                        ps[mg * 64 : (mg + 1) * 64, :],
                        lhsT=vt[kg * 64 : (kg + 1) * 64, off : off + 64],
                        rhs=h_tiles[q // 2][kg * 64 : (kg + 1) * 64, :],
                        start=bstart,
                        stop=bstop,
                        tile_position=(kg * 64, mg * 64),
                        skip_group_check=True,
                    )
                    if sync_after:
                        # Completion fence between this row's two chains:
                        # DVE reads the bank (waits chain1 completion);
                        # chain2's first MM gets a WAR dep on this read,
                        # so the two chains can never overlap in time
                        # (same PSUM bank from different PE row-groups
                        # would be a fatal collision).
                        nc.vector.tensor_copy(
                            sct[mg * 64 : mg * 64 + 64, :],
                            ps[mg * 64 : (mg + 1) * 64, 0:8],
                        )
                    if bstop:
                        p = int(row_pos[r])
                        assert p % 2 == mg
                        nc.scalar.activation(
                            h2_tiles[p // 2][mg * 64 : (mg + 1) * 64, :],
                            ps[mg * 64 : (mg + 1) * 64, :],
                            Relu,
                        )


def _build_body(nc, tc, xp, w1p, b1p, v2p, w3p, b3p, yp, BSH, D_IN, H, D_OUT, sched, DT, phases="ABC"):
    KI, MH, MO = D_IN // 128, H // 128, D_OUT // 128
    f32 = mybir.dt.float32
    Relu = mybir.ActivationFunctionType.Relu
    Ident = mybir.ActivationFunctionType.Identity
    is_ladder = sched["scheme"] == "ladder"
    S = sched["S"] if is_ladder else None
    with (
        tc.tile_pool(name="consts", bufs=1) as constp,
        tc.tile_pool(name="h2pool", bufs=1) as h2pool,
        tc.tile_pool(name="v2pool", bufs=4) as v2pool,
        tc.tile_pool(name="w3pool", bufs=3) as w3pool,
        tc.tile_pool(name="ypool", bufs=2) as ypool,
    ):
        # v2/w3/y pools are allocated up front (disjoint SBUF from the
        # phase-A pools) so their DMAs can prefetch during earlier phases
        # instead of stalling the PE at each phase boundary.
        v2_tiles = {}

        def fetch_strip(si):
            s0 = si * FC2_STRIP
            w = min(FC2_STRIP, S - s0)
            vt = v2pool.tile([128, w * 128], DT, tag="v2", name=f"v2s{si}")
            nc.sync.dma_start(out=vt[:], in_=v2p[:, s0 * 128 : (s0 + w) * 128])
            v2_tiles[si] = vt

        w3_tiles = {}

        def fetch_w3(mo):
            wt = w3pool.tile([128, H], DT, tag="w3", name=f"w3t{mo}")
            nc.sync.dma_start(out=wt[:], in_=w3p[:, mo * H : (mo + 1) * H])
            w3_tiles[mo] = wt

        b1t = constp.tile([128, MH], f32)
        nc.sync.dma_start(out=b1t[:], in_=b1p[:, :])
        b3t = constp.tile([128, MO], f32)
        nc.sync.dma_start(out=b3t[:], in_=b3p[:, :])

        with tc.tile_pool(name="hpool", bufs=1) as hpool:
            h_tiles = []
            # ---- Phase A: hT = relu(W1 @ xT + b1) ----
            # v2 strips for early fc2 slots are DMA'd during the tail of
            # phase A (after the corresponding late w1 tiles so they don't
            # head-block the w1 stream on the DMA queue).
            n_strips = (S + FC2_STRIP - 1) // FC2_STRIP if is_ladder else 0
            strip_at = {}
            for j in range(min(4, n_strips)):
                strip_at.setdefault(max(0, MH - 8 + 2 * j), []).append(j)
            KC = KI // 2  # x is DMA'd in two chunks so fc1 starts sooner
            with (
                tc.tile_pool(name="xpool", bufs=1) as xpool,
                tc.tile_pool(name="w1pool", bufs=4) as w1pool,
                tc.tile_pool(name="psumA", bufs=4, space="PSUM") as psumA,
            ):
                xts = []
                xt0 = xpool.tile([128, KC * BSH], DT, tag="x0", name="xt0")
                nc.sync.dma_start(out=xt0[:], in_=xp[:, 0 : KC * BSH])
                for mt in range(MH):
                    wt = w1pool.tile([128, D_IN], DT, tag="w1")
                    nc.sync.dma_start(
                        out=wt[:], in_=w1p[:, mt * D_IN : (mt + 1) * D_IN]
                    )
                    if mt == 0:
                        xt1 = xpool.tile([128, (KI - KC) * BSH], DT, tag="x1", name="xt1")
                        nc.sync.dma_start(out=xt1[:], in_=xp[:, KC * BSH :])
                        xts = [xt0, xt1]
                    for j in strip_at.get(mt, ()):
                        fetch_strip(j)
                    ps = psumA.tile([128, BSH], f32, tag="ps")
                    for n in range(KI):
                        xt = xts[n // KC]
                        nc.tensor.matmul(
                            ps[:],
                            lhsT=wt[:, n * 128 : (n + 1) * 128],
                            rhs=xt[:, (n % KC) * BSH : (n % KC + 1) * BSH],
                            start=(n == 0),
                            stop=(n == KI - 1),
                        )
                    ht = hpool.tile([128, BSH], DT, tag=f"h{mt}")
                    nc.scalar.activation(
                        ht[:], ps[:], Relu, bias=b1t[:, mt : mt + 1]
                    )
                    h_tiles.append(ht)

            if "B" not in phases:
                # timing probe: flush last h tile so phase A isn't dead
                nc.sync.dma_start(
                    out=yp[:, 0 : BSH // 2], in_=h_tiles[-1][:].bitcast(f32)
                )
                return
            # ---- Phase B: h2T = relu(W2_bsr @ hT) ----
            # first w3 tiles prefetch during fc2
            fetch_w3(0)
            fetch_w3(1)
            h2_tiles = [
                h2pool.tile([128, BSH], DT, tag=f"h2_{i}", name=f"h2_{i}")
                for i in range(MH)
            ]
            if sched["scheme"] == "ladder":
                _fc2_ladder(
                    nc, tc, v2p, h_tiles, h2_tiles, sched, BSH, DT,
                    v2_tiles, fetch_strip,
                )
            else:
                groups = sched["groups"]
                with (
                    tc.tile_pool(name="v2pool", bufs=3) as v2pool,
                    tc.tile_pool(name="psumB", bufs=4, space="PSUM") as psumB,
                ):
                    g0 = 0
                    for r2, lst in enumerate(groups):
                        ng = len(lst)
                        vt = v2pool.tile([128, ng * 128], DT, tag="v2")
                        nc.sync.dma_start(
                            out=vt[:], in_=v2p[:, g0 * 128 : (g0 + ng) * 128]
                        )
                        ps = psumB.tile([128, BSH], f32, tag="ps")
                        for j, t in enumerate(lst):
                            nc.tensor.matmul(
                                ps[:],
                                lhsT=vt[:, j * 128 : (j + 1) * 128],
                                rhs=h_tiles[t][:],
                                start=(j == 0),
                                stop=(j == ng - 1),
                            )
                        nc.scalar.activation(h2_tiles[r2][:], ps[:], Relu)
                        g0 += ng

        if "C" not in phases:
            nc.sync.dma_start(
                out=yp[:, 0 : BSH // 2], in_=h2_tiles[-1][:].bitcast(f32)
            )
            return
        # ---- Phase C: yT = W3 @ h2T + b3 ----
        # each output tile DMAs out (on the Scalar HWDGE queue, so the
        # stores never head-block the Sync queue's weight prefetches) as
        # soon as its activation retires; only the last tile's store is
        # exposed in the tail. Remaining w3 fetches are all issued up
        # front — their pool-ring waits pace them.
        with tc.tile_pool(name="psumC", bufs=4, space="PSUM") as psumC:
            for mo in range(2, MO):
                fetch_w3(mo)
            for mo in range(MO):
                wt = w3_tiles.pop(mo)
                ps = psumC.tile([128, BSH], f32, tag="ps")
                for k in range(MH):
                    nc.tensor.matmul(
                        ps[:],
                        lhsT=wt[:, k * 128 : (k + 1) * 128],
                        rhs=h2_tiles[k][:],
                        start=(k == 0),
                        stop=(k == MH - 1),
                    )
                yt = ypool.tile([128, BSH], f32, tag="yt", name=f"yt{mo}")
                nc.scalar.activation(
                    yt[:],
                    ps[:],
                    Ident,
                    bias=b3t[:, mo : mo + 1],
                )
                nc.scalar.dma_start(
                    out=yp[:, mo * BSH : (mo + 1) * BSH], in_=yt[:]
                )


# =====================================================================
# Host packing / run
# =====================================================================


def pack_inputs(
    x, w1, b1, values, w3, b3, crow, col, mode, n_cores=N_CORES, scheme=SCHEME,
    use_quad=False,
):
    """Host-side swizzle of all tensors into the DRAM layouts build_nc
    expects. Returns (shared_map, per_core_xp, sched, None)."""
    B, D_IN = x.shape
    H = w1.shape[0]
    D_OUT = w3.shape[0]
    KI, MH, MO = D_IN // 128, H // 128, D_OUT // 128
    BSH = B // n_cores
    store_np = _np_dt(mybir.dt.bfloat16) if mode == "bf16" else np.float32

    nbr = H // BS
    if scheme == "ladder":
        sched = build_ladder(crow, col, nbr)
        v2p = pack_v2_ladder(values, sched, store_np)
    else:
        sched = build_groups(crow, col, nbr)
        v2p = pack_v2_groups(values, sched, store_np)
    prow, pcol = sched["prow"], sched["pcol"]

    # fc1 output rows (= fc2 input block-cols) permuted by pcol;
    # fc3 contraction cols (= fc2 output block-rows) permuted by prow.
    w1 = w1.reshape(nbr, BS, D_IN)[pcol].reshape(H, D_IN)
    b1 = b1.reshape(nbr, BS)[pcol].reshape(H)
    w3 = w3.reshape(D_OUT, nbr, BS)[:, prow].reshape(D_OUT, H)

    w1p = np.ascontiguousarray(
        w1.reshape(MH, 128, KI, 128).transpose(3, 0, 2, 1).reshape(128, MH * D_IN)
    ).astype(store_np)
    w3p = np.ascontiguousarray(
        w3.reshape(MO, 128, MH, 128).transpose(3, 0, 2, 1).reshape(128, MO * H)
    ).astype(store_np)
    b1p = np.ascontiguousarray(b1.reshape(MH, 128).T).astype(np.float32)
    b3p = np.ascontiguousarray(b3.reshape(MO, 128).T).astype(np.float32)

    shared = {"w1p": w1p, "b1p": b1p, "v2p": v2p, "w3p": w3p, "b3p": b3p}
    xps = []
    for c in range(n_cores):
        xs = x[c * BSH : (c + 1) * BSH]
        xps.append(
            np.ascontiguousarray(
                xs.reshape(BSH, KI, 128).transpose(2, 1, 0).reshape(128, KI * BSH)
            ).astype(store_np)
        )
    return shared, xps, sched, None


def unpack_output(yps, B, D_OUT, n_cores=N_CORES):
    BSH = B // n_cores
    MO = D_OUT // 128
    out = np.empty((B, D_OUT), np.float32)
    for c, yp in enumerate(yps):
        out[c * BSH : (c + 1) * BSH] = (
            yp.reshape(128, MO, BSH).transpose(2, 1, 0).reshape(BSH, MO * 128)
        )
    return out


def run(x, w1, b1, values, w3, b3, crow, col, mode=MM_MODE, scheme=SCHEME, trace=False):
    B, D_IN = x.shape
    H = w1.shape[0]
    D_OUT = w3.shape[0]
    BSH = B // N_CORES
    shared, xps, sched, _ = pack_inputs(
        x, w1, b1, values, w3, b3, crow, col, mode, scheme=scheme
    )
    nc = build_nc(BSH, D_IN, H, D_OUT, sched, mode)
    in_maps = [dict(shared, xp=xps[c]) for c in range(N_CORES)]
    res = run_bass_kernel_spmd(nc, in_maps, core_ids=list(range(N_CORES)), trace=trace)
    out = unpack_output([res.results[c]["yp"] for c in range(N_CORES)], B, D_OUT)
    return out, res


def kernel(x, w1, b1, values, w3, b3, crow_indices, col_indices):
    x = np.asarray(x, np.float32)
    w1 = np.asarray(w1, np.float32)
    b1 = np.asarray(b1, np.float32)
    values = np.asarray(values, np.float32)
    w3 = np.asarray(w3, np.float32)
    b3 = np.asarray(b3, np.float32)
    crow = np.asarray(crow_indices)
    col = np.asarray(col_indices)
    out, _ = run(x, w1, b1, values, w3, b3, crow, col)
    return out



# revision 16
# speedup vs baseline: 1.1080x; 1.0456x over previous
"""Trainium2 Bass kernel for BlockedMLP:
    h1 = relu(x @ w1.T + b1)            # dense fc1
    h2 = relu(bsr_linear(h1, W2_bsr))   # 64x64-blocked sparse fc2
    y  = h2 @ w3.T + b3                 # dense fc3

Strategy: data-parallel over the batch dim across 8 NeuronCores
(weights replicated, no collectives). Everything is computed in a
feature-major ("transposed") layout so matmuls contract over the
partition dim with N = Bsh = 512 batch columns:

    hT  [H, Bsh]   = W1 @ xT      (Bsh = 4096/8 = 512 batch rows/core)
    h2T [H, Bsh]   = W2 @ hT      (BSR: compile-time-known sparsity)
    yT  [Dout,Bsh] = W3 @ h2T

fc1/fc3 are dense 128x128xK matmul chains at full PE rate. fc2 uses the
"ladder" scheme: the PE array is split into 4 concurrent 64x64 tiles
via tile_position. Quadrant (kg, mg) = (rhs partition half, psum/out
partition half). Each of the 64 BSR block rows is assigned an output
half mg and accumulates ALL its blocks into one half PSUM bank via two
sequential chains (cols with parity kg=0, then kg=1 — order can differ
per row), so no cross-bank merge is needed. A host-side balance anneal
picks the col parity classes + row mg split so all 4 quadrant lanes
carry exactly nnz/4 blocks, and a bounded-open-rows greedy open-shop
schedule achieves the optimal makespan (528 slots for the canonical
mask) with at most 3 rows' PSUM banks open per mg (8 banks total incl.
double buffering). fc2 thus runs at the MAC-optimal PE cycle count:
nnz/4 * 512 cycles, ~1.6x fewer tensor cycles than a 128x128 2x2-group
formulation.

All tensors are stored/streamed as bf16 (same PE throughput as f32r,
half the HBM traffic; PSUM accumulation stays fp32).
"""

import numpy as np

import concourse.bass as bass
import concourse.bacc as bacc
import concourse.mybir as mybir
from concourse import tile
from concourse.bass_utils import run_bass_kernel_spmd

BS = 64  # BSR block size
N_CORES = 8

# matmul dtype mode: "f32" | "f32r" | "bf16"
MM_MODE = "bf16"
# fc2 scheme: "ladder" (64x64 quadrant open-shop) | "groups" (2x2 128x128)
SCHEME = "ladder"
CAP_OPEN = 3  # max concurrently-accumulating rows per mg (PSUM pressure)


def _np_dt(dt):
    return mybir.dt.np(dt)


def _mask_sig(mask):
    import hashlib

    return hashlib.sha256(np.packbits(mask.astype(bool)).tobytes()).hexdigest()[:16]


# =====================================================================
# Ladder schedule construction (host side)
# =====================================================================

# Precomputed for the canonical BSR mask this problem generates
# (np.random.default_rng(0), density 0.5, col 0 forced): col parity and
# row mg assignments with perfectly balanced lane loads (528 each).
_KNOWN_SIG = "25b40de11a15c565"
_KNOWN_PAR_HEX = "569d112bb4765ae8"  # jitter-scanned greedy reaches optimal 528
_KNOWN_MG_HEX = "ce939ef9dd0001ba"


def _bits_to_hex(bits):
    return np.packbits(np.asarray(bits, np.uint8)).tobytes().hex()


def _hex_to_bits(h, n=64):
    return np.unpackbits(np.frombuffer(bytes.fromhex(h), np.uint8))[:n].astype(np.int8)


def _lane_loads(mask, par, mg):
    L = np.stack([mask @ (par == 0), mask @ (par == 1)], 1).astype(np.int64)
    loads = np.zeros((2, 2), np.int64)
    for m in (0, 1):
        sel = mg == m
        loads[0, m] = L[sel, 0].sum()
        loads[1, m] = L[sel, 1].sum()
    return loads, L


def anneal_balance(mask, iters=80000, seed=0):
    """Choose col parity (32/32) and row mg (32/32) minimizing the max
    quadrant-lane load max_{kg,mg} sum_{r in mg} L[r,kg]."""
    rng = np.random.default_rng(seed)
    n = mask.shape[0]
    par = np.zeros(n, np.int8)
    par[rng.permutation(n)[: n // 2]] = 1
    mg = np.zeros(n, np.int8)
    mg[rng.permutation(n)[: n // 2]] = 1
    total = int(mask.sum())
    ideal = (total + 3) // 4

    def cost(par, mg):
        loads, _ = _lane_loads(mask, par, mg)
        return int(loads.max()) * 10000 + int((loads.astype(float) ** 2).sum() / 100)

    cur = cost(par, mg)
    best, best_state = cur, (par.copy(), mg.copy())
    for it in range(iters):
        T = 2000.0 * (1.0 / 2000.0) ** (it / max(1, iters - 1))
        which = rng.random() < 0.5
        if which:
            c0, c1 = rng.integers(n, size=2)
            if par[c0] == par[c1]:
                continue
            par[c0], par[c1] = par[c1], par[c0]
        else:
            r0, r1 = rng.integers(n, size=2)
            if mg[r0] == mg[r1]:
                continue
            mg[r0], mg[r1] = mg[r1], mg[r0]
        new = cost(par, mg)
        if new <= cur or rng.random() < np.exp((cur - new) / T):
            cur = new
            if new < best:
                best, best_state = new, (par.copy(), mg.copy())
                if best // 10000 <= ideal:
                    break
        else:
            if which:
                par[c0], par[c1] = par[c1], par[c0]
            else:
                mg[r0], mg[r1] = mg[r1], mg[r0]
    return best_state


GAP = 3       # min slots between a row's chain1 end and chain2 start
              # (hides the DVE sync read latency)
RING = 4      # PSUM bank ring depth per mg (2 tags x 4 bufs = 8 banks)
MERGE_SLOTS = 2  # slots after chain2 end until the bank is drained (ACT)


def greedy_open_shop(L, rows, cap=CAP_OPEN, gap=GAP, ring=RING, jitter_seed=None):
    """Schedule rows (jobs with ops L[r,0] on lane0 / L[r,1] on lane1,
    ops non-overlapping per row with >= gap slots between them, <= cap
    rows open, bank ring reuse distance `ring`) on 2 lanes.
    Returns (makespan, seq) with seq[k] = list of (start, row, ln)."""
    T = [0, 0]
    pending = []  # (ready_time, row, lane, ln) second ops
    unopened = list(rows)
    seq = {0: [], 1: []}
    n_done, n = 0, len(rows)
    open_order = []   # rows in bank-allocation order
    bank_free = {}    # row -> time its bank is drained
    if jitter_seed is None:
        jit = {r: 0.0 for r in rows}
    else:
        u = np.random.default_rng(jitter_seed).random(len(rows))
        jit = {r: float(u[i]) for i, r in enumerate(rows)}
    guard = 0
    while n_done < n:
        guard += 1
        assert guard < 100000, "greedy_open_shop failed to converge"
        k = 0 if T[0] <= T[1] else 1
        ko = 1 - k
        cands = sorted(p for p in pending if p[2] == k)
        can_open = bool(unopened) and len(pending) < cap
        if can_open and len(open_order) >= ring:
            prev = open_order[len(open_order) - ring]
            if bank_free.get(prev, None) is None or bank_free[prev] > T[k]:
                can_open = False
        did = False
        if cands:
            rt, r, _, ln = cands[0]
            if rt <= T[k] or not can_open:
                start = max(T[k], rt)
                if ln > 0:
                    seq[k].append((start, r, ln))
                    T[k] = start + ln
                    bank_free[r] = T[k] + MERGE_SLOTS
                else:
                    bank_free[r] = start + MERGE_SLOTS
                pending.remove((rt, r, k, ln))
                n_done += 1
                did = True
        if not did and can_open:
            unopened.sort(
                key=lambda r: -(int(L[r, k]) - int(L[r, ko])) - jit[r]
            )
            r = unopened.pop(0)
            ln = int(L[r, k])
            open_order.append(r)
            if ln > 0:
                seq[k].append((T[k], r, ln))
                T[k] += ln
                pending.append((T[k] + gap, r, ko, int(L[r, ko])))
            else:
                pending.append((T[k], r, ko, int(L[r, ko])))
            did = True
        if not did:
            nxt = min([p[0] for p in pending], default=T[ko])
            T[k] = max(T[k] + 1, nxt)
    return max(T), seq


def build_ladder(crow, col, nbr, cap=CAP_OPEN):
    """Build the full fc2 ladder schedule. Returns a dict with:
      S: number of slots
      lanes[(kg,mg)]: length-S list of None | (row, col, first, last)
      prow, pcol: position -> physical row/col permutations
      row_pos, col_pos: physical -> position
      mg_of_row, par_of_col
    """
    mask = np.zeros((nbr, nbr), np.int64)
    for br in range(nbr):
        for idx in range(int(crow[br]), int(crow[br + 1])):
            mask[br, int(col[idx])] = 1

    sig = _mask_sig(mask)
    if nbr == 64 and sig == _KNOWN_SIG and _KNOWN_PAR_HEX:
        par = _hex_to_bits(_KNOWN_PAR_HEX, nbr)
        mg = _hex_to_bits(_KNOWN_MG_HEX, nbr)
    else:
        # try a few anneal seeds, keep the one with best greedy makespan
        best = None
        for seed in range(4):
            p2, m2 = anneal_balance(mask, iters=60000, seed=seed)
            _, L2 = _lane_loads(mask, p2, m2)
            span = max(
                greedy_open_shop(
                    L2, [r for r in range(nbr) if m2[r] == m], cap=cap
                )[0]
                for m in (0, 1)
            )
            if best is None or span < best[0]:
                best = (span, p2, m2)
        par, mg = best[1], best[2]

    loads, L = _lane_loads(mask, par, mg)

    lanes = {}
    closure = {}
    first_lane = {}
    spans = []
    seqs = {}
    for m in (0, 1):
        rows = [r for r in range(nbr) if mg[r] == m]
        best = None
        for js in [None] + list(range(200)):
            span, seq = greedy_open_shop(L, rows, cap=cap, jitter_seed=js)
            if best is None or span < best[0]:
                best = (span, seq)
        spans.append(best[0])
        seqs[m] = best[1]
    S = max(spans)
    for m in (0, 1):
        seq = seqs[m]
        # determine op order per row (seq only contains nonzero ops)
        ops = {}
        for k in (0, 1):
            for start, r, ln in seq[k]:
                ops.setdefault(r, []).append((start, k, ln))
        for r, lst in ops.items():
            lst.sort()
            first_lane[r] = lst[0][1]
            closure[r] = lst[-1][0] + lst[-1][2]
        for k in (0, 1):
            lane = [None] * S
            for start, r, ln in seq[k]:
                cols = sorted(np.nonzero(mask[r] * (par == k))[0].tolist())
                assert len(cols) == ln
                is_first_op = (ops[r][0][1] == k) and (ops[r][0][0] == start)
                is_last_op = (ops[r][-1][1] == k) and (ops[r][-1][0] == start)
                for j in range(ln):
                    # (row, col, bass_start, bass_stop, sync_after)
                    lane[start + j] = (
                        r,
                        cols[j],
                        is_first_op and j == 0,
                        is_last_op and j == ln - 1,
                        (not is_last_op) and j == ln - 1,
                    )
            lanes[(k, m)] = lane

    # positions: mg0 rows -> even positions by closure order; mg1 -> odd
    prow = np.zeros(nbr, np.int64)
    row_pos = np.zeros(nbr, np.int64)
    for m in (0, 1):
        rows = [r for r in range(nbr) if mg[r] == m]
        rows.sort(key=lambda r: (closure[r], r))
        for i, r in enumerate(rows):
            p = 2 * i + m
            prow[p] = r
            row_pos[r] = p
    pcol = np.zeros(nbr, np.int64)
    col_pos = np.zeros(nbr, np.int64)
    for k in (0, 1):
        cols = [c for c in range(nbr) if par[c] == k]
        for i, c in enumerate(cols):
            q = 2 * i + k
            pcol[q] = c
            col_pos[c] = q

    # block index lookup
    bidx = {}
    for br in range(nbr):
        for idx in range(int(crow[br]), int(crow[br + 1])):
            bidx[(br, int(col[idx]))] = idx

    n_mm = sum(
        1 for ln in lanes.values() for e in ln if e is not None
    )
    assert n_mm == int(mask.sum()), (n_mm, int(mask.sum()))

    return {
        "scheme": "ladder",
        "S": S,
        "lanes": lanes,
        "prow": prow,
        "pcol": pcol,
        "row_pos": row_pos,
        "col_pos": col_pos,
        "mg": mg,
        "par": par,
        "bidx": bidx,
        "nbr": nbr,
    }


def pack_v2_ladder(values, sched, store_np):
    """Pack fc2 blocks into [128, S*128]: slot s holds the 4 quadrant
    blocks: (kg,mg) at [kg*64:(kg+1)*64, s*128+mg*64 : s*128+mg*64+64],
    laid out as lhsT (block.T)."""
    S = sched["S"]
    lanes = sched["lanes"]
    bidx = sched["bidx"]
    v2 = np.zeros((128, S * 128), np.float32)
    for (kg, mg), lane in lanes.items():
        for s, e in enumerate(lane):
            if e is None:
                continue
            r, c = e[0], e[1]
            v2[
                kg * 64 : (kg + 1) * 64,
                s * 128 + mg * 64 : s * 128 + mg * 64 + 64,
            ] = values[bidx[(r, c)]].T
    return np.ascontiguousarray(v2.astype(store_np))


# =====================================================================
# Legacy 2x2-group scheme (kept for A/B benchmarking)
# =====================================================================

_KNOWN_PR = [52, 37, 12, 42, 35, 11, 27, 50, 33, 17, 38, 30, 1, 40, 21, 26, 14, 44, 63, 19, 18, 59, 24, 60, 43, 55, 0, 54, 28, 7, 8, 22, 20, 25, 61, 13, 34, 32, 51, 57, 36, 49, 31, 47, 2, 15, 39, 41, 58, 9, 56, 6, 16, 45, 62, 5, 10, 48, 3, 53, 46, 29, 4, 23]
_KNOWN_PC = [6, 51, 49, 33, 8, 22, 1, 18, 13, 50, 21, 5, 15, 0, 2, 25, 52, 41, 38, 9, 7, 37, 4, 63, 3, 14, 20, 60, 62, 35, 61, 17, 57, 11, 39, 34, 19, 58, 46, 54, 23, 16, 42, 30, 28, 12, 36, 32, 24, 47, 43, 59, 53, 27, 26, 40, 55, 10, 29, 45, 44, 48, 31, 56]


def optimize_pairing(mask, iters=60000, rounds=4, seed=0):
    rng = np.random.default_rng(seed)
    nr, nc = mask.shape
    prow = list(range(nr))
    pcol = list(range(nc))

    def anneal(perm, bits, iters):
        n = len(perm)

        def paircost(i):
            return (bits[perm[2 * i]] | bits[perm[2 * i + 1]]).bit_count()

        cost = [paircost(i) for i in range(n // 2)]
        u = rng.random(iters)
        idx = rng.integers(0, n, (iters, 2))
        T0, T1 = 1.5, 0.02
        for it in range(iters):
            a, b = idx[it]
            ia, ib = a // 2, b // 2
            if ia == ib:
                continue
            perm[a], perm[b] = perm[b], perm[a]
            na, nb = paircost(ia), paircost(ib)
            d = na + nb - cost[ia] - cost[ib]
            T = T0 * (T1 / T0) ** (it / iters)
            if d <= 0 or u[it] < np.exp(-d / T):
                cost[ia], cost[ib] = na, nb
            else:
                perm[a], perm[b] = perm[b], perm[a]

    for _ in range(rounds):
        rowbits = [
            int.from_bytes(
                np.packbits(
                    (mask[r, pcol].reshape(nc // 2, 2).any(axis=1)), bitorder="little"
                ).tobytes(),
                "little",
            )
            for r in range(nr)
        ]
        anneal(prow, rowbits, iters)
        colbits = [
            int.from_bytes(
                np.packbits(
                    (mask[prow, c].reshape(nr // 2, 2).any(axis=1)), bitorder="little"
                ).tobytes(),
                "little",
            )
            for c in range(nc)
        ]
        anneal(pcol, colbits, iters)
    return np.array(prow), np.array(pcol)


def build_groups(crow, col, nbr):
    blocks = {}
    mask = np.zeros((nbr, nbr), bool)
    for br in range(nbr):
        for idx in range(int(crow[br]), int(crow[br + 1])):
            c = int(col[idx])
            blocks[(br, c)] = idx
            mask[br, c] = True
    if mask.shape == (64, 64) and _mask_sig(mask) == _KNOWN_SIG:
        prow, pcol = np.array(_KNOWN_PR), np.array(_KNOWN_PC)
    else:
        prow, pcol = optimize_pairing(mask)
    pblocks = {}
    for i in range(nbr):
        for j in range(nbr):
            idx = blocks.get((int(prow[i]), int(pcol[j])))
            if idx is not None:
                pblocks[(i, j)] = idx
    R2 = nbr // 2
    groups = []
    for r2 in range(R2):
        lst = []
        for t in range(R2):
            if any(
                (2 * r2 + ir, 2 * t + ic) in pblocks for ir in (0, 1) for ic in (0, 1)
            ):
                lst.append(t)
        groups.append(lst)
    return {
        "scheme": "groups",
        "groups": groups,
        "pblocks": pblocks,
        "prow": prow,
        "pcol": pcol,
        "nbr": nbr,
    }


def pack_v2_groups(values, sched, store_np):
    groups, blocks = sched["groups"], sched["pblocks"]
    G = sum(len(g) for g in groups)
    v2 = np.zeros((128, G * 128), np.float32)
    g = 0
    for r2, lst in enumerate(groups):
        for t in lst:
            Z = np.zeros((128, 128), np.float32)
            for ir in (0, 1):
                for ic in (0, 1):
                    idx = blocks.get((2 * r2 + ir, 2 * t + ic))
                    if idx is not None:
                        Z[ic * 64 : ic * 64 + 64, ir * 64 : ir * 64 + 64] = values[
                            idx
                        ].T
            v2[:, g * 128 : (g + 1) * 128] = Z
            g += 1
    return np.ascontiguousarray(v2.astype(store_np))


# =====================================================================
# Bass program
# =====================================================================


def build_nc(BSH, D_IN, H, D_OUT, sched, mode, repeat=1, quad=None, phases="ABC",
             warmup=True):
    """Build the per-core Bass program (SPMD: same program on all cores)."""
    KI, MH, MO = D_IN // 128, H // 128, D_OUT // 128
    if sched["scheme"] == "ladder":
        G = sched["S"]
    else:
        G = sum(len(g) for g in sched["groups"])
    f32 = mybir.dt.float32
    if mode == "bf16":
        DT = mybir.dt.bfloat16
    elif mode == "f32r":
        DT = mybir.dt.float32r
    else:
        DT = f32

    # All large tensors are laid out as per-tile-contiguous DRAM slabs
    # ([ntiles*128, tile_cols]; a DMA partition-slice reads one fully
    # contiguous region) — 2KB-per-row strided reads run well below peak
    # HBM bandwidth and starved the w1 stream in earlier revisions.
    n_strips = (G + FC2_STRIP - 1) // FC2_STRIP
    KC = KI // 2
    nc = bacc.Bacc(None, target_bir_lowering=False)
    xp = nc.declare_dram_parameter("xp", [2 * 128, KC * BSH], DT, isOutput=False)
    w1p = nc.declare_dram_parameter("w1p", [MH * 128, D_IN], DT, isOutput=False)
    b1p = nc.declare_dram_parameter("b1p", [128, MH], f32, isOutput=False)
    v2p = nc.declare_dram_parameter(
        "v2p", [n_strips * 128, FC2_STRIP * 128], DT, isOutput=False
    )
    w3p = nc.declare_dram_parameter("w3p", [MO * 128, H], DT, isOutput=False)
    b3p = nc.declare_dram_parameter("b3p", [128, MO], f32, isOutput=False)
    yp = nc.declare_dram_parameter("yp", [MO * 128, BSH], f32, isOutput=True)

    with tile.TileContext(nc) as tc:
        if warmup and DT != f32:
            # One-time PE warm-up: ~8 dependency-free matmuls on a zeroed
            # tile keep TensorE busy through the HAM activity window while
            # the first input DMAs are in flight, so the real fc1 matmuls
            # start at 2.4 GHz instead of 1.2 GHz. Outside the repeat loop
            # (cost amortizes to ~0 in steady state).
            with (
                tc.tile_pool(name="warm", bufs=1) as wp,
                tc.tile_pool(name="warmps", bufs=1, space="PSUM") as wpp,
            ):
                zt = wp.tile([128, BSH], DT, name="warmz")
                nc.vector.memset(zt[:], 0.0)
                wps = wpp.tile([128, BSH], f32, name="warmp")
                for _ in range(8):
                    nc.tensor.matmul(
                        wps[:], lhsT=zt[:, 0:128], rhs=zt[:], start=True, stop=True
                    )
        for _rep in range(repeat):
            _build_body(
                nc, tc, xp, w1p, b1p, v2p, w3p, b3p, yp, BSH, D_IN, H, D_OUT,
                sched, DT, phases=phases,
            )
    nc.compile()
    return nc


FC2_STRIP = 48  # fc2 v2 DMA strip width (slots)


def _fc2_ladder(nc, tc, v2p, h_tiles, h2_tiles, sched, BSH, DT, v2_tiles,
                fetch_strip, qpool):
    """Emit the ladder-scheme fc2: 4 concurrent 64x64 quadrant lanes,
    per-row half-bank accumulation, ACT relu on retire. v2_tiles holds
    strips already prefetched during phase A; fetch_strip(si) DMAs the
    rest as their ring slots free up. qpool is the shared all-phase PSUM
    pool (tags q0/q1)."""
    f32 = mybir.dt.float32
    Relu = mybir.ActivationFunctionType.Relu
    S = sched["S"]
    lanes = sched["lanes"]
    row_pos = sched["row_pos"]
    col_pos = sched["col_pos"]
    STRIP = FC2_STRIP
    row_psum = {}
    with tc.tile_pool(name="scratch", bufs=1) as scpool:
        sct = scpool.tile([128, 8], f32, name="sct")
        for s0 in range(0, S, STRIP):
            si = s0 // STRIP
            w = min(STRIP, S - s0)
            if si not in v2_tiles:
                fetch_strip(si)
            vt = v2_tiles.pop(si)
            for s in range(s0, s0 + w):
                for kg, mg in ((0, 0), (1, 0), (0, 1), (1, 1)):
                    e = lanes[(kg, mg)][s]
                    if e is None:
                        continue
                    r, c, bstart, bstop, sync_after = e
                    if bstart:
                        row_psum[r] = qpool.tile(
                            [128, BSH], f32, tag=f"q{mg}", name=f"q_{r}"
                        )
                    ps = row_psum[r]
                    q = int(col_pos[c])
                    off = (s - s0) * 128 + mg * 64
                    nc.tensor.matmul(
                        ps[mg * 64 : (mg + 1) * 64, :],
                        lhsT=vt[kg * 64 : (kg + 1) * 64, off : off + 64],
                        rhs=h_tiles[q // 2][kg * 64 : (kg + 1) * 64, :],
                        start=bstart,
                        stop=bstop,
                        tile_position=(kg * 64, mg * 64),
                        skip_group_check=True,
                    )
                    if sync_after:
                        # Completion fence between this row's two chains:
                        # DVE reads the bank (waits chain1 completion);
                        # chain2's first MM gets a WAR dep on this read,
                        # so the two chains can never overlap in time
                        # (same PSUM bank from different PE row-groups
                        # would be a fatal collision).
                        nc.vector.tensor_copy(
                            sct[mg * 64 : mg * 64 + 64, :],
                            ps[mg * 64 : (mg + 1) * 64, 0:8],
                        )
                    if bstop:
                        p = int(row_pos[r])
                        assert p % 2 == mg
                        nc.scalar.activation(
                            h2_tiles[p // 2][mg * 64 : (mg + 1) * 64, :],
                            ps[mg * 64 : (mg + 1) * 64, :],
                            Relu,
                        )


def _build_body(nc, tc, xp, w1p, b1p, v2p, w3p, b3p, yp, BSH, D_IN, H, D_OUT, sched, DT, phases="ABC"):
    KI, MH, MO = D_IN // 128, H // 128, D_OUT // 128
    f32 = mybir.dt.float32
    Relu = mybir.ActivationFunctionType.Relu
    Ident = mybir.ActivationFunctionType.Identity
    is_ladder = sched["scheme"] == "ladder"
    S = sched["S"] if is_ladder else None
    with (
        tc.tile_pool(name="consts", bufs=1) as constp,
        tc.tile_pool(name="h2pool", bufs=1) as h2pool,
        tc.tile_pool(name="v2pool", bufs=4) as v2pool,
        tc.tile_pool(name="w3pool", bufs=3) as w3pool,
        tc.tile_pool(name="ypool", bufs=2) as ypool,
        tc.tile_pool(name="qpsum", bufs=4, space="PSUM") as qpool,
    ):
        # v2/w3/y pools are allocated up front (disjoint SBUF from the
        # phase-A pools) so their DMAs can prefetch during earlier phases
        # instead of stalling the PE at each phase boundary. qpsum (2 tags
        # x 4 bufs = all 8 banks) is shared by all three phases: a fresh
        # chain only waits on its own ring slot's drain instead of a
        # pool-boundary barrier against every bank.
        v2_tiles = {}

        def fetch_strip(si):
            vt = v2pool.tile([128, FC2_STRIP * 128], DT, tag="v2", name=f"v2s{si}")
            nc.sync.dma_start(out=vt[:], in_=v2p[si * 128 : (si + 1) * 128, :])
            v2_tiles[si] = vt

        w3_tiles = {}

        def fetch_w3(mo):
            wt = w3pool.tile([128, H], DT, tag="w3", name=f"w3t{mo}")
            nc.sync.dma_start(out=wt[:], in_=w3p[mo * 128 : (mo + 1) * 128, :])
            w3_tiles[mo] = wt

        b1t = constp.tile([128, MH], f32)
        nc.sync.dma_start(out=b1t[:], in_=b1p[:, :])
        b3t = constp.tile([128, MO], f32)
        nc.sync.dma_start(out=b3t[:], in_=b3p[:, :])

        with tc.tile_pool(name="hpool", bufs=1) as hpool:
            h_tiles = []
            # ---- Phase A: hT = relu(W1 @ xT + b1) ----
            # v2 strips for early fc2 slots are DMA'd during the tail of
            # phase A (after the corresponding late w1 tiles so they don't
            # head-block the w1 stream on the DMA queue).
            n_strips = (S + FC2_STRIP - 1) // FC2_STRIP if is_ladder else 0
            strip_at = {4: [0], 10: [1], 16: [2], 22: [3]}
            KC = KI // 2  # x is DMA'd in two chunks so fc1 starts sooner
            with (
                tc.tile_pool(name="xpool", bufs=1) as xpool,
                tc.tile_pool(name="w1pool", bufs=4) as w1pool,
            ):
                xts = []
                xt0 = xpool.tile([128, KC * BSH], DT, tag="x0", name="xt0")
                nc.sync.dma_start(out=xt0[:], in_=xp[0:128, :])
                for mt in range(MH):
                    wt = w1pool.tile([128, D_IN], DT, tag="w1")
                    nc.sync.dma_start(
                        out=wt[:], in_=w1p[mt * 128 : (mt + 1) * 128, :]
                    )
                    if mt == 0:
                        xt1 = xpool.tile([128, KC * BSH], DT, tag="x1", name="xt1")
                        nc.sync.dma_start(out=xt1[:], in_=xp[128:256, :])
                        xts = [xt0, xt1]
                    for j in strip_at.get(mt, ()) if is_ladder else ():
                        fetch_strip(j)
                    ps = qpool.tile(
                        [128, BSH], f32, tag=f"q{mt % 2}", name=f"psA{mt}"
                    )
                    for n in range(KI):
                        xt = xts[n // KC]
                        nc.tensor.matmul(
                            ps[:],
                            lhsT=wt[:, n * 128 : (n + 1) * 128],
                            rhs=xt[:, (n % KC) * BSH : (n % KC + 1) * BSH],
                            start=(n == 0),
                            stop=(n == KI - 1),
                        )
                    ht = hpool.tile([128, BSH], DT, tag=f"h{mt}")
                    nc.scalar.activation(
                        ht[:], ps[:], Relu, bias=b1t[:, mt : mt + 1]
                    )
                    h_tiles.append(ht)

            if "B" not in phases:
                # timing probe: flush last h tile so phase A isn't dead
                nc.sync.dma_start(
                    out=yp[:, 0 : BSH // 2], in_=h_tiles[-1][:].bitcast(f32)
                )
                return
            # ---- Phase B: h2T = relu(W2_bsr @ hT) ----
            # first w3 tiles prefetch during fc2
            fetch_w3(0)
            fetch_w3(1)
            h2_tiles = [
                h2pool.tile([128, BSH], DT, tag=f"h2_{i}", name=f"h2_{i}")
                for i in range(MH)
            ]
            if sched["scheme"] == "ladder":
                _fc2_ladder(
                    nc, tc, v2p, h_tiles, h2_tiles, sched, BSH, DT,
                    v2_tiles, fetch_strip, qpool,
                )
            else:
                groups = sched["groups"]
                with (
                    tc.tile_pool(name="v2pool", bufs=3) as v2pool,
                    tc.tile_pool(name="psumB", bufs=4, space="PSUM") as psumB,
                ):
                    g0 = 0
                    for r2, lst in enumerate(groups):
                        ng = len(lst)
                        vt = v2pool.tile([128, ng * 128], DT, tag="v2")
                        nc.sync.dma_start(
                            out=vt[:], in_=v2p[:, g0 * 128 : (g0 + ng) * 128]
                        )
                        ps = psumB.tile([128, BSH], f32, tag="ps")
                        for j, t in enumerate(lst):
                            nc.tensor.matmul(
                                ps[:],
                                lhsT=vt[:, j * 128 : (j + 1) * 128],
                                rhs=h_tiles[t][:],
                                start=(j == 0),
                                stop=(j == ng - 1),
                            )
                        nc.scalar.activation(h2_tiles[r2][:], ps[:], Relu)
                        g0 += ng

        if "C" not in phases:
            nc.sync.dma_start(
                out=yp[:, 0 : BSH // 2], in_=h2_tiles[-1][:].bitcast(f32)
            )
            return
        # ---- Phase C: yT = W3 @ h2T + b3 ----
        # each output tile DMAs out (on the Scalar HWDGE queue, so the
        # stores never head-block the Sync queue's weight prefetches) as
        # soon as its activation retires; only the last tile's store is
        # exposed in the tail. Remaining w3 fetches are all issued up
        # front — their pool-ring waits pace them.
        for mo in range(2, MO):
            fetch_w3(mo)
        for mo in range(MO):
            wt = w3_tiles.pop(mo)
            ps = qpool.tile([128, BSH], f32, tag=f"q{mo % 2}", name=f"psC{mo}")
            for k in range(MH):
                nc.tensor.matmul(
                    ps[:],
                    lhsT=wt[:, k * 128 : (k + 1) * 128],
                    rhs=h2_tiles[k][:],
                    start=(k == 0),
                    stop=(k == MH - 1),
                )
            yt = ypool.tile([128, BSH], f32, tag="yt", name=f"yt{mo}")
            nc.scalar.activation(
                yt[:],
                ps[:],
                Ident,
                bias=b3t[:, mo : mo + 1],
            )
            nc.scalar.dma_start(
                out=yp[mo * 128 : (mo + 1) * 128, :], in_=yt[:]
            )


# =====================================================================
# Host packing / run
# =====================================================================


def pack_inputs(
    x, w1, b1, values, w3, b3, crow, col, mode, n_cores=N_CORES, scheme=SCHEME,
    use_quad=False,
):
    """Host-side swizzle of all tensors into the DRAM layouts build_nc
    expects. Returns (shared_map, per_core_xp, sched, None)."""
    B, D_IN = x.shape
    H = w1.shape[0]
    D_OUT = w3.shape[0]
    KI, MH, MO = D_IN // 128, H // 128, D_OUT // 128
    BSH = B // n_cores
    store_np = _np_dt(mybir.dt.bfloat16) if mode == "bf16" else np.float32

    nbr = H // BS
    if scheme == "ladder":
        sched = build_ladder(crow, col, nbr)
        v2p = pack_v2_ladder(values, sched, store_np)
    else:
        sched = build_groups(crow, col, nbr)
        v2p = pack_v2_groups(values, sched, store_np)
    prow, pcol = sched["prow"], sched["pcol"]

    # fc1 output rows (= fc2 input block-cols) permuted by pcol;
    # fc3 contraction cols (= fc2 output block-rows) permuted by prow.
    w1 = w1.reshape(nbr, BS, D_IN)[pcol].reshape(H, D_IN)
    b1 = b1.reshape(nbr, BS)[pcol].reshape(H)
    w3 = w3.reshape(D_OUT, nbr, BS)[:, prow].reshape(D_OUT, H)

    # Per-tile-contiguous DRAM slabs: slab[t*128 + p, c] = tile t's lhsT
    # laid out so a DMA partition-slice is one contiguous read.
    w1p = np.ascontiguousarray(
        w1.reshape(MH, 128, KI, 128).transpose(0, 3, 2, 1).reshape(MH * 128, D_IN)
    ).astype(store_np)
    w3p = np.ascontiguousarray(
        w3.reshape(MO, 128, MH, 128).transpose(0, 3, 2, 1).reshape(MO * 128, H)
    ).astype(store_np)
    b1p = np.ascontiguousarray(b1.reshape(MH, 128).T).astype(np.float32)
    b3p = np.ascontiguousarray(b3.reshape(MO, 128).T).astype(np.float32)

    # v2 slab: strip si -> rows [si*128, (si+1)*128)
    S_cols = v2p.shape[1]  # S * 128
    S = S_cols // 128
    n_strips = (S + FC2_STRIP - 1) // FC2_STRIP
    v2pad = np.zeros((128, n_strips * FC2_STRIP * 128), v2p.dtype)
    v2pad[:, :S_cols] = v2p
    v2p = np.ascontiguousarray(
        v2pad.reshape(128, n_strips, FC2_STRIP * 128)
        .transpose(1, 0, 2)
        .reshape(n_strips * 128, FC2_STRIP * 128)
    )

    shared = {"w1p": w1p, "b1p": b1p, "v2p": v2p, "w3p": w3p, "b3p": b3p}
    xps = []
    KC = KI // 2
    for c in range(n_cores):
        xs = x[c * BSH : (c + 1) * BSH]
        xps.append(
            np.ascontiguousarray(
                xs.reshape(BSH, 2, KC, 128)
                .transpose(1, 3, 2, 0)
                .reshape(2 * 128, KC * BSH)
            ).astype(store_np)
        )
    return shared, xps, sched, None


def unpack_output(yps, B, D_OUT, n_cores=N_CORES):
    BSH = B // n_cores
    MO = D_OUT // 128
    out = np.empty((B, D_OUT), np.float32)
    for c, yp in enumerate(yps):
        out[c * BSH : (c + 1) * BSH] = (
            yp.reshape(MO, 128, BSH).transpose(2, 0, 1).reshape(BSH, MO * 128)
        )
    return out


def run(x, w1, b1, values, w3, b3, crow, col, mode=MM_MODE, scheme=SCHEME, trace=False):
    B, D_IN = x.shape
    H = w1.shape[0]
    D_OUT = w3.shape[0]
    BSH = B // N_CORES
    shared, xps, sched, _ = pack_inputs(
        x, w1, b1, values, w3, b3, crow, col, mode, scheme=scheme
    )
    nc = build_nc(BSH, D_IN, H, D_OUT, sched, mode)
    in_maps = [dict(shared, xp=xps[c]) for c in range(N_CORES)]
    res = run_bass_kernel_spmd(nc, in_maps, core_ids=list(range(N_CORES)), trace=trace)
    out = unpack_output([res.results[c]["yp"] for c in range(N_CORES)], B, D_OUT)
    return out, res


def kernel(x, w1, b1, values, w3, b3, crow_indices, col_indices):
    x = np.asarray(x, np.float32)
    w1 = np.asarray(w1, np.float32)
    b1 = np.asarray(b1, np.float32)
    values = np.asarray(values, np.float32)
    w3 = np.asarray(w3, np.float32)
    b3 = np.asarray(b3, np.float32)
    crow = np.asarray(crow_indices)
    col = np.asarray(col_indices)
    out, _ = run(x, w1, b1, values, w3, b3, crow, col)
    return out



# revision 18
# speedup vs baseline: 1.1290x; 1.0190x over previous
"""Trainium2 Bass kernel for BlockedMLP:
    h1 = relu(x @ w1.T + b1)            # dense fc1
    h2 = relu(bsr_linear(h1, W2_bsr))   # 64x64-blocked sparse fc2
    y  = h2 @ w3.T + b3                 # dense fc3

Strategy: data-parallel over the batch dim across 8 NeuronCores
(weights replicated, no collectives). Everything is computed in a
feature-major ("transposed") layout so matmuls contract over the
partition dim with N = Bsh = 512 batch columns:

    hT  [H, Bsh]   = W1 @ xT      (Bsh = 4096/8 = 512 batch rows/core)
    h2T [H, Bsh]   = W2 @ hT      (BSR: compile-time-known sparsity)
    yT  [Dout,Bsh] = W3 @ h2T

fc1/fc3 are dense 128x128xK matmul chains at full PE rate. fc2 uses the
"ladder" scheme: the PE array is split into 4 concurrent 64x64 tiles
via tile_position. Quadrant (kg, mg) = (rhs partition half, psum/out
partition half). Each of the 64 BSR block rows is assigned an output
half mg and accumulates ALL its blocks into one half PSUM bank via two
sequential chains (cols with parity kg=0, then kg=1 — order can differ
per row), so no cross-bank merge is needed. A host-side balance anneal
picks the col parity classes + row mg split so all 4 quadrant lanes
carry exactly nnz/4 blocks, and a bounded-open-rows greedy open-shop
schedule achieves the optimal makespan (528 slots for the canonical
mask) with at most 3 rows' PSUM banks open per mg (8 banks total incl.
double buffering). fc2 thus runs at the MAC-optimal PE cycle count:
nnz/4 * 512 cycles, ~1.6x fewer tensor cycles than a 128x128 2x2-group
formulation.

All tensors are stored/streamed as bf16 (same PE throughput as f32r,
half the HBM traffic; PSUM accumulation stays fp32).
"""

import numpy as np

import concourse.bass as bass
import concourse.bacc as bacc
import concourse.mybir as mybir
from concourse import tile
from concourse.bass_utils import run_bass_kernel_spmd

BS = 64  # BSR block size
N_CORES = 8

# matmul dtype mode: "f32" | "f32r" | "bf16"
MM_MODE = "bf16"
# fc2 scheme: "ladder" (64x64 quadrant open-shop) | "groups" (2x2 128x128)
SCHEME = "ladder"
CAP_OPEN = 3  # max concurrently-accumulating rows per mg (PSUM pressure)


def _np_dt(dt):
    return mybir.dt.np(dt)


def _mask_sig(mask):
    import hashlib

    return hashlib.sha256(np.packbits(mask.astype(bool)).tobytes()).hexdigest()[:16]


# =====================================================================
# Ladder schedule construction (host side)
# =====================================================================

# Precomputed for the canonical BSR mask this problem generates
# (np.random.default_rng(0), density 0.5, col 0 forced): col parity and
# row mg assignments with perfectly balanced lane loads (528 each).
_KNOWN_SIG = "25b40de11a15c565"
_KNOWN_PAR_HEX = "569d112bb4765ae8"  # jitter-scanned greedy reaches optimal 528
_KNOWN_MG_HEX = "ce939ef9dd0001ba"


def _bits_to_hex(bits):
    return np.packbits(np.asarray(bits, np.uint8)).tobytes().hex()


def _hex_to_bits(h, n=64):
    return np.unpackbits(np.frombuffer(bytes.fromhex(h), np.uint8))[:n].astype(np.int8)


def _lane_loads(mask, par, mg):
    L = np.stack([mask @ (par == 0), mask @ (par == 1)], 1).astype(np.int64)
    loads = np.zeros((2, 2), np.int64)
    for m in (0, 1):
        sel = mg == m
        loads[0, m] = L[sel, 0].sum()
        loads[1, m] = L[sel, 1].sum()
    return loads, L


def anneal_balance(mask, iters=80000, seed=0):
    """Choose col parity (32/32) and row mg (32/32) minimizing the max
    quadrant-lane load max_{kg,mg} sum_{r in mg} L[r,kg]."""
    rng = np.random.default_rng(seed)
    n = mask.shape[0]
    par = np.zeros(n, np.int8)
    par[rng.permutation(n)[: n // 2]] = 1
    mg = np.zeros(n, np.int8)
    mg[rng.permutation(n)[: n // 2]] = 1
    total = int(mask.sum())
    ideal = (total + 3) // 4

    def cost(par, mg):
        loads, _ = _lane_loads(mask, par, mg)
        return int(loads.max()) * 10000 + int((loads.astype(float) ** 2).sum() / 100)

    cur = cost(par, mg)
    best, best_state = cur, (par.copy(), mg.copy())
    for it in range(iters):
        T = 2000.0 * (1.0 / 2000.0) ** (it / max(1, iters - 1))
        which = rng.random() < 0.5
        if which:
            c0, c1 = rng.integers(n, size=2)
            if par[c0] == par[c1]:
                continue
            par[c0], par[c1] = par[c1], par[c0]
        else:
            r0, r1 = rng.integers(n, size=2)
            if mg[r0] == mg[r1]:
                continue
            mg[r0], mg[r1] = mg[r1], mg[r0]
        new = cost(par, mg)
        if new <= cur or rng.random() < np.exp((cur - new) / T):
            cur = new
            if new < best:
                best, best_state = new, (par.copy(), mg.copy())
                if best // 10000 <= ideal:
                    break
        else:
            if which:
                par[c0], par[c1] = par[c1], par[c0]
            else:
                mg[r0], mg[r1] = mg[r1], mg[r0]
    return best_state


GAP = 3       # min slots between a row's chain1 end and chain2 start
              # (hides the DVE sync read latency)
RING = 4      # PSUM bank ring depth per mg (2 tags x 4 bufs = 8 banks)
MERGE_SLOTS = 2  # slots after chain2 end until the bank is drained (ACT)


def greedy_open_shop(L, rows, cap=CAP_OPEN, gap=GAP, ring=RING, jitter_seed=None):
    """Schedule rows (jobs with ops L[r,0] on lane0 / L[r,1] on lane1,
    ops non-overlapping per row with >= gap slots between them, <= cap
    rows open, bank ring reuse distance `ring`) on 2 lanes.
    Returns (makespan, seq) with seq[k] = list of (start, row, ln)."""
    T = [0, 0]
    pending = []  # (ready_time, row, lane, ln) second ops
    unopened = list(rows)
    seq = {0: [], 1: []}
    n_done, n = 0, len(rows)
    open_order = []   # rows in bank-allocation order
    bank_free = {}    # row -> time its bank is drained
    if jitter_seed is None:
        jit = {r: 0.0 for r in rows}
    else:
        u = np.random.default_rng(jitter_seed).random(len(rows))
        jit = {r: float(u[i]) for i, r in enumerate(rows)}
    guard = 0
    while n_done < n:
        guard += 1
        assert guard < 100000, "greedy_open_shop failed to converge"
        k = 0 if T[0] <= T[1] else 1
        ko = 1 - k
        cands = sorted(p for p in pending if p[2] == k)
        can_open = bool(unopened) and len(pending) < cap
        if can_open and len(open_order) >= ring:
            prev = open_order[len(open_order) - ring]
            if bank_free.get(prev, None) is None or bank_free[prev] > T[k]:
                can_open = False
        did = False
        if cands:
            rt, r, _, ln = cands[0]
            if rt <= T[k] or not can_open:
                start = max(T[k], rt)
                if ln > 0:
                    seq[k].append((start, r, ln))
                    T[k] = start + ln
                    bank_free[r] = T[k] + MERGE_SLOTS
                else:
                    bank_free[r] = start + MERGE_SLOTS
                pending.remove((rt, r, k, ln))
                n_done += 1
                did = True
        if not did and can_open:
            unopened.sort(
                key=lambda r: -(int(L[r, k]) - int(L[r, ko])) - jit[r]
            )
            r = unopened.pop(0)
            ln = int(L[r, k])
            open_order.append(r)
            if ln > 0:
                seq[k].append((T[k], r, ln))
                T[k] += ln
                pending.append((T[k] + gap, r, ko, int(L[r, ko])))
            else:
                pending.append((T[k], r, ko, int(L[r, ko])))
            did = True
        if not did:
            nxt = min([p[0] for p in pending], default=T[ko])
            T[k] = max(T[k] + 1, nxt)
    return max(T), seq


def build_ladder(crow, col, nbr, cap=CAP_OPEN):
    """Build the full fc2 ladder schedule. Returns a dict with:
      S: number of slots
      lanes[(kg,mg)]: length-S list of None | (row, col, first, last)
      prow, pcol: position -> physical row/col permutations
      row_pos, col_pos: physical -> position
      mg_of_row, par_of_col
    """
    mask = np.zeros((nbr, nbr), np.int64)
    for br in range(nbr):
        for idx in range(int(crow[br]), int(crow[br + 1])):
            mask[br, int(col[idx])] = 1

    sig = _mask_sig(mask)
    if nbr == 64 and sig == _KNOWN_SIG and _KNOWN_PAR_HEX:
        par = _hex_to_bits(_KNOWN_PAR_HEX, nbr)
        mg = _hex_to_bits(_KNOWN_MG_HEX, nbr)
    else:
        # try a few anneal seeds, keep the one with best greedy makespan
        best = None
        for seed in range(4):
            p2, m2 = anneal_balance(mask, iters=60000, seed=seed)
            _, L2 = _lane_loads(mask, p2, m2)
            span = max(
                greedy_open_shop(
                    L2, [r for r in range(nbr) if m2[r] == m], cap=cap
                )[0]
                for m in (0, 1)
            )
            if best is None or span < best[0]:
                best = (span, p2, m2)
        par, mg = best[1], best[2]

    loads, L = _lane_loads(mask, par, mg)

    lanes = {}
    closure = {}
    first_lane = {}
    spans = []
    seqs = {}
    for m in (0, 1):
        rows = [r for r in range(nbr) if mg[r] == m]
        best = None
        for js in [None] + list(range(200)):
            span, seq = greedy_open_shop(L, rows, cap=cap, jitter_seed=js)
            if best is None or span < best[0]:
                best = (span, seq)
        spans.append(best[0])
        seqs[m] = best[1]
    S = max(spans)
    for m in (0, 1):
        seq = seqs[m]
        # determine op order per row (seq only contains nonzero ops)
        ops = {}
        for k in (0, 1):
            for start, r, ln in seq[k]:
                ops.setdefault(r, []).append((start, k, ln))
        for r, lst in ops.items():
            lst.sort()
            first_lane[r] = lst[0][1]
            closure[r] = lst[-1][0] + lst[-1][2]
        for k in (0, 1):
            lane = [None] * S
            for start, r, ln in seq[k]:
                cols = sorted(np.nonzero(mask[r] * (par == k))[0].tolist())
                assert len(cols) == ln
                is_first_op = (ops[r][0][1] == k) and (ops[r][0][0] == start)
                is_last_op = (ops[r][-1][1] == k) and (ops[r][-1][0] == start)
                for j in range(ln):
                    # (row, col, bass_start, bass_stop, sync_after)
                    lane[start + j] = (
                        r,
                        cols[j],
                        is_first_op and j == 0,
                        is_last_op and j == ln - 1,
                        (not is_last_op) and j == ln - 1,
                    )
            lanes[(k, m)] = lane

    # positions: mg0 rows -> even positions by closure order; mg1 -> odd
    prow = np.zeros(nbr, np.int64)
    row_pos = np.zeros(nbr, np.int64)
    for m in (0, 1):
        rows = [r for r in range(nbr) if mg[r] == m]
        rows.sort(key=lambda r: (closure[r], r))
        for i, r in enumerate(rows):
            p = 2 * i + m
            prow[p] = r
            row_pos[r] = p
    pcol = np.zeros(nbr, np.int64)
    col_pos = np.zeros(nbr, np.int64)
    for k in (0, 1):
        cols = [c for c in range(nbr) if par[c] == k]
        for i, c in enumerate(cols):
            q = 2 * i + k
            pcol[q] = c
            col_pos[c] = q

    # block index lookup
    bidx = {}
    for br in range(nbr):
        for idx in range(int(crow[br]), int(crow[br + 1])):
            bidx[(br, int(col[idx]))] = idx

    n_mm = sum(
        1 for ln in lanes.values() for e in ln if e is not None
    )
    assert n_mm == int(mask.sum()), (n_mm, int(mask.sum()))

    return {
        "scheme": "ladder",
        "S": S,
        "lanes": lanes,
        "prow": prow,
        "pcol": pcol,
        "row_pos": row_pos,
        "col_pos": col_pos,
        "mg": mg,
        "par": par,
        "bidx": bidx,
        "nbr": nbr,
    }


def pack_v2_ladder(values, sched, store_np):
    """Pack fc2 blocks into [128, S*128]: slot s holds the 4 quadrant
    blocks: (kg,mg) at [kg*64:(kg+1)*64, s*128+mg*64 : s*128+mg*64+64],
    laid out as lhsT (block.T)."""
    S = sched["S"]
    lanes = sched["lanes"]
    bidx = sched["bidx"]
    v2 = np.zeros((128, S * 128), np.float32)
    for (kg, mg), lane in lanes.items():
        for s, e in enumerate(lane):
            if e is None:
                continue
            r, c = e[0], e[1]
            v2[
                kg * 64 : (kg + 1) * 64,
                s * 128 + mg * 64 : s * 128 + mg * 64 + 64,
            ] = values[bidx[(r, c)]].T
    return np.ascontiguousarray(v2.astype(store_np))


# =====================================================================
# Legacy 2x2-group scheme (kept for A/B benchmarking)
# =====================================================================

_KNOWN_PR = [52, 37, 12, 42, 35, 11, 27, 50, 33, 17, 38, 30, 1, 40, 21, 26, 14, 44, 63, 19, 18, 59, 24, 60, 43, 55, 0, 54, 28, 7, 8, 22, 20, 25, 61, 13, 34, 32, 51, 57, 36, 49, 31, 47, 2, 15, 39, 41, 58, 9, 56, 6, 16, 45, 62, 5, 10, 48, 3, 53, 46, 29, 4, 23]
_KNOWN_PC = [6, 51, 49, 33, 8, 22, 1, 18, 13, 50, 21, 5, 15, 0, 2, 25, 52, 41, 38, 9, 7, 37, 4, 63, 3, 14, 20, 60, 62, 35, 61, 17, 57, 11, 39, 34, 19, 58, 46, 54, 23, 16, 42, 30, 28, 12, 36, 32, 24, 47, 43, 59, 53, 27, 26, 40, 55, 10, 29, 45, 44, 48, 31, 56]


def optimize_pairing(mask, iters=60000, rounds=4, seed=0):
    rng = np.random.default_rng(seed)
    nr, nc = mask.shape
    prow = list(range(nr))
    pcol = list(range(nc))

    def anneal(perm, bits, iters):
        n = len(perm)

        def paircost(i):
            return (bits[perm[2 * i]] | bits[perm[2 * i + 1]]).bit_count()

        cost = [paircost(i) for i in range(n // 2)]
        u = rng.random(iters)
        idx = rng.integers(0, n, (iters, 2))
        T0, T1 = 1.5, 0.02
        for it in range(iters):
            a, b = idx[it]
            ia, ib = a // 2, b // 2
            if ia == ib:
                continue
            perm[a], perm[b] = perm[b], perm[a]
            na, nb = paircost(ia), paircost(ib)
            d = na + nb - cost[ia] - cost[ib]
            T = T0 * (T1 / T0) ** (it / iters)
            if d <= 0 or u[it] < np.exp(-d / T):
                cost[ia], cost[ib] = na, nb
            else:
                perm[a], perm[b] = perm[b], perm[a]

    for _ in range(rounds):
        rowbits = [
            int.from_bytes(
                np.packbits(
                    (mask[r, pcol].reshape(nc // 2, 2).any(axis=1)), bitorder="little"
                ).tobytes(),
                "little",
            )
            for r in range(nr)
        ]
        anneal(prow, rowbits, iters)
        colbits = [
            int.from_bytes(
                np.packbits(
                    (mask[prow, c].reshape(nr // 2, 2).any(axis=1)), bitorder="little"
                ).tobytes(),
                "little",
            )
            for c in range(nc)
        ]
        anneal(pcol, colbits, iters)
    return np.array(prow), np.array(pcol)


def build_groups(crow, col, nbr):
    blocks = {}
    mask = np.zeros((nbr, nbr), bool)
    for br in range(nbr):
        for idx in range(int(crow[br]), int(crow[br + 1])):
            c = int(col[idx])
            blocks[(br, c)] = idx
            mask[br, c] = True
    if mask.shape == (64, 64) and _mask_sig(mask) == _KNOWN_SIG:
        prow, pcol = np.array(_KNOWN_PR), np.array(_KNOWN_PC)
    else:
        prow, pcol = optimize_pairing(mask)
    pblocks = {}
    for i in range(nbr):
        for j in range(nbr):
            idx = blocks.get((int(prow[i]), int(pcol[j])))
            if idx is not None:
                pblocks[(i, j)] = idx
    R2 = nbr // 2
    groups = []
    for r2 in range(R2):
        lst = []
        for t in range(R2):
            if any(
                (2 * r2 + ir, 2 * t + ic) in pblocks for ir in (0, 1) for ic in (0, 1)
            ):
                lst.append(t)
        groups.append(lst)
    return {
        "scheme": "groups",
        "groups": groups,
        "pblocks": pblocks,
        "prow": prow,
        "pcol": pcol,
        "nbr": nbr,
    }


def pack_v2_groups(values, sched, store_np):
    groups, blocks = sched["groups"], sched["pblocks"]
    G = sum(len(g) for g in groups)
    v2 = np.zeros((128, G * 128), np.float32)
    g = 0
    for r2, lst in enumerate(groups):
        for t in lst:
            Z = np.zeros((128, 128), np.float32)
            for ir in (0, 1):
                for ic in (0, 1):
                    idx = blocks.get((2 * r2 + ir, 2 * t + ic))
                    if idx is not None:
                        Z[ic * 64 : ic * 64 + 64, ir * 64 : ir * 64 + 64] = values[
                            idx
                        ].T
            v2[:, g * 128 : (g + 1) * 128] = Z
            g += 1
    return np.ascontiguousarray(v2.astype(store_np))


# =====================================================================
# Bass program
# =====================================================================


def build_nc(BSH, D_IN, H, D_OUT, sched, mode, repeat=1, quad=None, phases="ABC",
             warmup=True):
    """Build the per-core Bass program (SPMD: same program on all cores)."""
    KI, MH, MO = D_IN // 128, H // 128, D_OUT // 128
    if sched["scheme"] == "ladder":
        G = sched["S"]
    else:
        G = sum(len(g) for g in sched["groups"])
    f32 = mybir.dt.float32
    if mode == "bf16":
        DT = mybir.dt.bfloat16
    elif mode == "f32r":
        DT = mybir.dt.float32r
    else:
        DT = f32

    # All large tensors are laid out as per-tile-contiguous DRAM slabs
    # ([ntiles*128, tile_cols]; a DMA partition-slice reads one fully
    # contiguous region) — 2KB-per-row strided reads run well below peak
    # HBM bandwidth and starved the w1 stream in earlier revisions.
    n_strips = (G + FC2_STRIP - 1) // FC2_STRIP
    KC = KI // 2
    nc = bacc.Bacc(None, target_bir_lowering=False)
    xp = nc.declare_dram_parameter("xp", [2 * 128, KC * BSH], DT, isOutput=False)
    w1p = nc.declare_dram_parameter("w1p", [MH * 128, D_IN], DT, isOutput=False)
    b1p = nc.declare_dram_parameter("b1p", [128, MH], f32, isOutput=False)
    v2p = nc.declare_dram_parameter(
        "v2p", [n_strips * 128, FC2_STRIP * 128], DT, isOutput=False
    )
    w3p = nc.declare_dram_parameter("w3p", [MO * 128, H], DT, isOutput=False)
    b3p = nc.declare_dram_parameter("b3p", [128, MO], f32, isOutput=False)
    yp = nc.declare_dram_parameter("yp", [MO * 128, BSH], f32, isOutput=True)

    with tile.TileContext(nc) as tc:
        if warmup and DT != f32:
            # One-time PE warm-up: ~8 dependency-free matmuls on a zeroed
            # tile keep TensorE busy through the HAM activity window while
            # the first input DMAs are in flight, so the real fc1 matmuls
            # start at 2.4 GHz instead of 1.2 GHz. Outside the repeat loop
            # (cost amortizes to ~0 in steady state).
            with (
                tc.tile_pool(name="warm", bufs=1) as wp,
                tc.tile_pool(name="warmps", bufs=1, space="PSUM") as wpp,
            ):
                zt = wp.tile([128, BSH], DT, name="warmz")
                nc.vector.memset(zt[:], 0.0)
                wps = wpp.tile([128, BSH], f32, name="warmp")
                for _ in range(8):
                    nc.tensor.matmul(
                        wps[:], lhsT=zt[:, 0:128], rhs=zt[:], start=True, stop=True
                    )
        for _rep in range(repeat):
            _build_body(
                nc, tc, xp, w1p, b1p, v2p, w3p, b3p, yp, BSH, D_IN, H, D_OUT,
                sched, DT, phases=phases,
            )
    nc.compile()
    return nc


FC2_STRIP = 24  # fc2 v2 DMA strip width (slots); small enough that a
                # strip transfer burst (~2.2us) doesn't starve the w1
                # stream during phase A prefetch


def _fc2_ladder(nc, tc, v2p, h_tiles, h2_tiles, sched, BSH, DT, v2_tiles,
                fetch_strip, qpool):
    """Emit the ladder-scheme fc2: 4 concurrent 64x64 quadrant lanes,
    per-row half-bank accumulation, ACT relu on retire. v2_tiles holds
    strips already prefetched during phase A; fetch_strip(si) DMAs the
    rest as their ring slots free up. qpool is the shared all-phase PSUM
    pool (tags q0/q1)."""
    f32 = mybir.dt.float32
    Relu = mybir.ActivationFunctionType.Relu
    S = sched["S"]
    lanes = sched["lanes"]
    row_pos = sched["row_pos"]
    col_pos = sched["col_pos"]
    STRIP = FC2_STRIP
    row_psum = {}
    with tc.tile_pool(name="scratch", bufs=1) as scpool:
        sct = scpool.tile([128, 8], f32, name="sct")
        for s0 in range(0, S, STRIP):
            si = s0 // STRIP
            w = min(STRIP, S - s0)
            if si not in v2_tiles:
                fetch_strip(si)
            vt = v2_tiles.pop(si)
            for s in range(s0, s0 + w):
                for kg, mg in ((0, 0), (1, 0), (0, 1), (1, 1)):
                    e = lanes[(kg, mg)][s]
                    if e is None:
                        continue
                    r, c, bstart, bstop, sync_after = e
                    if bstart:
                        row_psum[r] = qpool.tile(
                            [128, BSH], f32, tag=f"q{mg}", name=f"q_{r}"
                        )
                    ps = row_psum[r]
                    q = int(col_pos[c])
                    off = (s - s0) * 128 + mg * 64
                    nc.tensor.matmul(
                        ps[mg * 64 : (mg + 1) * 64, :],
                        lhsT=vt[kg * 64 : (kg + 1) * 64, off : off + 64],
                        rhs=h_tiles[q // 2][kg * 64 : (kg + 1) * 64, :],
                        start=bstart,
                        stop=bstop,
                        tile_position=(kg * 64, mg * 64),
                        skip_group_check=True,
                    )
                    if sync_after:
                        # Completion fence between this row's two chains:
                        # DVE reads the bank (waits chain1 completion);
                        # chain2's first MM gets a WAR dep on this read,
                        # so the two chains can never overlap in time
                        # (same PSUM bank from different PE row-groups
                        # would be a fatal collision).
                        nc.vector.tensor_copy(
                            sct[mg * 64 : mg * 64 + 64, :],
                            ps[mg * 64 : (mg + 1) * 64, 0:8],
                        )
                    if bstop:
                        p = int(row_pos[r])
                        assert p % 2 == mg
                        nc.scalar.activation(
                            h2_tiles[p // 2][mg * 64 : (mg + 1) * 64, :],
                            ps[mg * 64 : (mg + 1) * 64, :],
                            Relu,
                        )


def _build_body(nc, tc, xp, w1p, b1p, v2p, w3p, b3p, yp, BSH, D_IN, H, D_OUT, sched, DT, phases="ABC"):
    KI, MH, MO = D_IN // 128, H // 128, D_OUT // 128
    f32 = mybir.dt.float32
    Relu = mybir.ActivationFunctionType.Relu
    Ident = mybir.ActivationFunctionType.Identity
    is_ladder = sched["scheme"] == "ladder"
    S = sched["S"] if is_ladder else None
    with (
        tc.tile_pool(name="consts", bufs=1) as constp,
        tc.tile_pool(name="h2pool", bufs=1) as h2pool,
        tc.tile_pool(name="v2pool", bufs=4) as v2pool,
        tc.tile_pool(name="w3pool", bufs=3) as w3pool,
        tc.tile_pool(name="ypool", bufs=2) as ypool,
        tc.tile_pool(name="qpsum", bufs=4, space="PSUM") as qpool,
    ):
        # v2/w3/y pools are allocated up front (disjoint SBUF from the
        # phase-A pools) so their DMAs can prefetch during earlier phases
        # instead of stalling the PE at each phase boundary. qpsum (2 tags
        # x 4 bufs = all 8 banks) is shared by all three phases: a fresh
        # chain only waits on its own ring slot's drain instead of a
        # pool-boundary barrier against every bank.
        v2_tiles = {}

        def fetch_strip(si):
            vt = v2pool.tile([128, FC2_STRIP * 128], DT, tag="v2", name=f"v2s{si}")
            nc.sync.dma_start(out=vt[:], in_=v2p[si * 128 : (si + 1) * 128, :])
            v2_tiles[si] = vt

        w3_tiles = {}

        def fetch_w3(mo):
            wt = w3pool.tile([128, H], DT, tag="w3", name=f"w3t{mo}")
            nc.sync.dma_start(out=wt[:], in_=w3p[mo * 128 : (mo + 1) * 128, :])
            w3_tiles[mo] = wt

        b1t = constp.tile([128, MH], f32)
        nc.sync.dma_start(out=b1t[:], in_=b1p[:, :])
        b3t = constp.tile([128, MO], f32)
        nc.sync.dma_start(out=b3t[:], in_=b3p[:, :])

        with tc.tile_pool(name="hpool", bufs=1) as hpool:
            h_tiles = []
            # ---- Phase A: hT = relu(W1 @ xT + b1) ----
            # v2 strips for early fc2 slots are DMA'd during the tail of
            # phase A (after the corresponding late w1 tiles so they don't
            # head-block the w1 stream on the DMA queue).
            n_strips = (S + FC2_STRIP - 1) // FC2_STRIP if is_ladder else 0
            strip_at = {6: [0], 12: [1], 18: [2], 24: [3]}
            KC = KI // 2  # x is DMA'd in two chunks so fc1 starts sooner
            with (
                tc.tile_pool(name="xpool", bufs=1) as xpool,
                tc.tile_pool(name="w1pool", bufs=4) as w1pool,
            ):
                xts = []
                xt0 = xpool.tile([128, KC * BSH], DT, tag="x0", name="xt0")
                nc.sync.dma_start(out=xt0[:], in_=xp[0:128, :])
                for mt in range(MH):
                    wt = w1pool.tile([128, D_IN], DT, tag="w1")
                    nc.sync.dma_start(
                        out=wt[:], in_=w1p[mt * 128 : (mt + 1) * 128, :]
                    )
                    if mt == 0:
                        xt1 = xpool.tile([128, KC * BSH], DT, tag="x1", name="xt1")
                        nc.sync.dma_start(out=xt1[:], in_=xp[128:256, :])
                        xts = [xt0, xt1]
                    for j in strip_at.get(mt, ()) if is_ladder else ():
                        fetch_strip(j)
                    ps = qpool.tile(
                        [128, BSH], f32, tag=f"q{mt % 2}", name=f"psA{mt}"
                    )
                    for n in range(KI):
                        xt = xts[n // KC]
                        nc.tensor.matmul(
                            ps[:],
                            lhsT=wt[:, n * 128 : (n + 1) * 128],
                            rhs=xt[:, (n % KC) * BSH : (n % KC + 1) * BSH],
                            start=(n == 0),
                            stop=(n == KI - 1),
                        )
                    ht = hpool.tile([128, BSH], DT, tag=f"h{mt}")
                    nc.scalar.activation(
                        ht[:], ps[:], Relu, bias=b1t[:, mt : mt + 1]
                    )
                    h_tiles.append(ht)

            if "B" not in phases:
                # timing probe: flush last h tile so phase A isn't dead
                nc.sync.dma_start(
                    out=yp[:, 0 : BSH // 2], in_=h_tiles[-1][:].bitcast(f32)
                )
                return
            # ---- Phase B: h2T = relu(W2_bsr @ hT) ----
            # first w3 tiles prefetch during fc2
            fetch_w3(0)
            fetch_w3(1)
            h2_tiles = [
                h2pool.tile([128, BSH], DT, tag=f"h2_{i}", name=f"h2_{i}")
                for i in range(MH)
            ]
            if sched["scheme"] == "ladder":
                _fc2_ladder(
                    nc, tc, v2p, h_tiles, h2_tiles, sched, BSH, DT,
                    v2_tiles, fetch_strip, qpool,
                )
            else:
                groups = sched["groups"]
                with (
                    tc.tile_pool(name="v2pool", bufs=3) as v2pool,
                    tc.tile_pool(name="psumB", bufs=4, space="PSUM") as psumB,
                ):
                    g0 = 0
                    for r2, lst in enumerate(groups):
                        ng = len(lst)
                        vt = v2pool.tile([128, ng * 128], DT, tag="v2")
                        nc.sync.dma_start(
                            out=vt[:], in_=v2p[:, g0 * 128 : (g0 + ng) * 128]
                        )
                        ps = psumB.tile([128, BSH], f32, tag="ps")
                        for j, t in enumerate(lst):
                            nc.tensor.matmul(
                                ps[:],
                                lhsT=vt[:, j * 128 : (j + 1) * 128],
                                rhs=h_tiles[t][:],
                                start=(j == 0),
                                stop=(j == ng - 1),
                            )
                        nc.scalar.activation(h2_tiles[r2][:], ps[:], Relu)
                        g0 += ng

        if "C" not in phases:
            nc.sync.dma_start(
                out=yp[:, 0 : BSH // 2], in_=h2_tiles[-1][:].bitcast(f32)
            )
            return
        # ---- Phase C: yT = W3 @ h2T + b3 ----
        # each output tile DMAs out (on the Scalar HWDGE queue, so the
        # stores never head-block the Sync queue's weight prefetches) as
        # soon as its activation retires; only the last tile's store is
        # exposed in the tail. Remaining w3 fetches are all issued up
        # front — their pool-ring waits pace them.
        for mo in range(2, MO):
            fetch_w3(mo)
        for mo in range(MO):
            wt = w3_tiles.pop(mo)
            ps = qpool.tile([128, BSH], f32, tag=f"q{mo % 2}", name=f"psC{mo}")
            for k in range(MH):
                nc.tensor.matmul(
                    ps[:],
                    lhsT=wt[:, k * 128 : (k + 1) * 128],
                    rhs=h2_tiles[k][:],
                    start=(k == 0),
                    stop=(k == MH - 1),
                )
            yt = ypool.tile([128, BSH], f32, tag="yt", name=f"yt{mo}")
            nc.scalar.activation(
                yt[:],
                ps[:],
                Ident,
                bias=b3t[:, mo : mo + 1],
            )
            nc.scalar.dma_start(
                out=yp[mo * 128 : (mo + 1) * 128, :], in_=yt[:]
            )


# =====================================================================
# Host packing / run
# =====================================================================


def pack_inputs(
    x, w1, b1, values, w3, b3, crow, col, mode, n_cores=N_CORES, scheme=SCHEME,
    use_quad=False,
):
    """Host-side swizzle of all tensors into the DRAM layouts build_nc
    expects. Returns (shared_map, per_core_xp, sched, None)."""
    B, D_IN = x.shape
    H = w1.shape[0]
    D_OUT = w3.shape[0]
    KI, MH, MO = D_IN // 128, H // 128, D_OUT // 128
    BSH = B // n_cores
    store_np = _np_dt(mybir.dt.bfloat16) if mode == "bf16" else np.float32

    nbr = H // BS
    if scheme == "ladder":
        sched = build_ladder(crow, col, nbr)
        v2p = pack_v2_ladder(values, sched, store_np)
    else:
        sched = build_groups(crow, col, nbr)
        v2p = pack_v2_groups(values, sched, store_np)
    prow, pcol = sched["prow"], sched["pcol"]

    # fc1 output rows (= fc2 input block-cols) permuted by pcol;
    # fc3 contraction cols (= fc2 output block-rows) permuted by prow.
    w1 = w1.reshape(nbr, BS, D_IN)[pcol].reshape(H, D_IN)
    b1 = b1.reshape(nbr, BS)[pcol].reshape(H)
    w3 = w3.reshape(D_OUT, nbr, BS)[:, prow].reshape(D_OUT, H)

    # Per-tile-contiguous DRAM slabs: slab[t*128 + p, c] = tile t's lhsT
    # laid out so a DMA partition-slice is one contiguous read.
    w1p = np.ascontiguousarray(
        w1.reshape(MH, 128, KI, 128).transpose(0, 3, 2, 1).reshape(MH * 128, D_IN)
    ).astype(store_np)
    w3p = np.ascontiguousarray(
        w3.reshape(MO, 128, MH, 128).transpose(0, 3, 2, 1).reshape(MO * 128, H)
    ).astype(store_np)
    b1p = np.ascontiguousarray(b1.reshape(MH, 128).T).astype(np.float32)
    b3p = np.ascontiguousarray(b3.reshape(MO, 128).T).astype(np.float32)

    # v2 slab: strip si -> rows [si*128, (si+1)*128)
    S_cols = v2p.shape[1]  # S * 128
    S = S_cols // 128
    n_strips = (S + FC2_STRIP - 1) // FC2_STRIP
    v2pad = np.zeros((128, n_strips * FC2_STRIP * 128), v2p.dtype)
    v2pad[:, :S_cols] = v2p
    v2p = np.ascontiguousarray(
        v2pad.reshape(128, n_strips, FC2_STRIP * 128)
        .transpose(1, 0, 2)
        .reshape(n_strips * 128, FC2_STRIP * 128)
    )

    shared = {"w1p": w1p, "b1p": b1p, "v2p": v2p, "w3p": w3p, "b3p": b3p}
    xps = []
    KC = KI // 2
    for c in range(n_cores):
        xs = x[c * BSH : (c + 1) * BSH]
        xps.append(
            np.ascontiguousarray(
                xs.reshape(BSH, 2, KC, 128)
                .transpose(1, 3, 2, 0)
                .reshape(2 * 128, KC * BSH)
            ).astype(store_np)
        )
    return shared, xps, sched, None


def unpack_output(yps, B, D_OUT, n_cores=N_CORES):
    BSH = B // n_cores
    MO = D_OUT // 128
    out = np.empty((B, D_OUT), np.float32)
    for c, yp in enumerate(yps):
        out[c * BSH : (c + 1) * BSH] = (
            yp.reshape(MO, 128, BSH).transpose(2, 0, 1).reshape(BSH, MO * 128)
        )
    return out


def run(x, w1, b1, values, w3, b3, crow, col, mode=MM_MODE, scheme=SCHEME, trace=False):
    B, D_IN = x.shape
    H = w1.shape[0]
    D_OUT = w3.shape[0]
    BSH = B // N_CORES
    shared, xps, sched, _ = pack_inputs(
        x, w1, b1, values, w3, b3, crow, col, mode, scheme=scheme
    )
    nc = build_nc(BSH, D_IN, H, D_OUT, sched, mode)
    in_maps = [dict(shared, xp=xps[c]) for c in range(N_CORES)]
    res = run_bass_kernel_spmd(nc, in_maps, core_ids=list(range(N_CORES)), trace=trace)
    out = unpack_output([res.results[c]["yp"] for c in range(N_CORES)], B, D_OUT)
    return out, res


def kernel(x, w1, b1, values, w3, b3, crow_indices, col_indices):
    x = np.asarray(x, np.float32)
    w1 = np.asarray(w1, np.float32)
    b1 = np.asarray(b1, np.float32)
    values = np.asarray(values, np.float32)
    w3 = np.asarray(w3, np.float32)
    b3 = np.asarray(b3, np.float32)
    crow = np.asarray(crow_indices)
    col = np.asarray(col_indices)
    out, _ = run(x, w1, b1, values, w3, b3, crow, col)
    return out

